# revision 52
# baseline (speedup 1.0000x reference)
"""GAT (3-layer) Trainium2 Bass kernel, 8-core SPMD.

Schedule:
 - Nodes are relabeled and packed by the host into a uniform schedule:
   8 cores x B blocks x 2 windows x 64 slots. Each (window, src-half) gets a
   fixed number S of 128-edge sub-chunks; all per-core variation lives in
   data (gather indices / dst-slot metadata), so one SPMD program serves all
   cores. Per layer: node phase (h = x @ Wc, AllGather of node tables),
   then edge phase (bulk dma_gather of h[src] rows, segment softmax +
   weighted sums via indicator matmuls accumulating in PSUM).

Data-plane design (end-to-end I/O is the dominant cost):
 - Node tables are [NPAD, 128] f16 (256B rows == minimum dma_gather
   granularity): row = h_tilde = (x @ W.T) * bn_scale (BN folded on host).
 - asrc is recomputed per edge on-chip: grouped reduce over the gathered
   h_tilde row against aW = a_src/bn_scale. No asrc in the table rows.
 - adst never hits DRAM: per block the 128 local-node adst values (kept in
   SBUF from the node phase) are spread to edge lanes with an
   indicator-transpose matmul, indT = is_equal(iota_p, dstlT).
 - Per sub-chunk ONE accumulation matmul: rhs = [h*ee | ee] so messages and
   softmax denominators accumulate together in PSUM.
 - All inputs ship as ONE packed blob per core (~1.6 MB): x quantized to
   10-bit (5 bytes per value quad, unpacked on-chip with shift/mask ops;
   the scale is folded into Wc0), de-replicated [16, W] int16 index
   streams (tiled to [128, W] by DMA), int8 dstl/dstlT, f16 weights.
 - Output is ONE int8 array per core: rows [q(40) | per-row f16 scale
   bitcast into the last two bytes], dequantized on host.
"""
import os
import numpy as np

os.environ.setdefault("JAX_COMPILATION_CACHE_DIR", "/tmp/jaxcache")

import concourse.bass as bass
import concourse.bacc as bacc
import concourse.tile as tile
import concourse.mybir as mybir
from concourse.bass_utils import run_bass_kernel_spmd
from concourse.masks import make_identity

P = 128
f32 = mybir.dt.float32
f16 = mybir.dt.float16
i16 = mybir.dt.int16
i8 = mybir.dt.int8
u8 = mybir.dt.uint8
u16 = mybir.dt.uint16

NEG_SLOPE = 0.2
BN_EPS = 1e-5
EL = 128            # f16 elems per table row (256B = min gather granularity)


class Cfg:
    def __init__(self, n, e, ncores=8, bpc=None, s=None, batch_blocks=4,
                 heads=8, ch=16, out_ch=40, in_ch=128):
        self.N = n
        self.E = e
        self.NCORES = ncores
        self.IN_CH = in_ch
        self.HID = heads * ch
        self.HEADS = heads
        self.CH = ch
        self.OUT_CH = out_ch
        assert n % ncores == 0
        self.npc_real = n // ncores
        self.BPC = bpc if bpc else (self.npc_real + P - 1) // P
        self.NPC = self.BPC * P              # node slots per core
        self.NPAD = self.NPC * ncores        # total node slots
        self.HALFN = self.NPAD // 2          # table half size
        assert ncores % 2 == 0
        assert self.HALFN < 32768, "half table must be int16 indexable"
        self.S = s
        bb = []
        nb = self.BPC
        while nb > 0:
            take = min(batch_blocks, nb)
            bb.append(take)
            nb -= take
        self.batches = bb


def _pack_core(deg_l, deg_h, nodes, bpc, cap):
    """Greedy 2D bin packing: nodes (orig ids) -> window. bins = bpc*2
    windows with 64 slots, capacity cap on both L and H edge sums."""
    nwin = bpc * 2
    rem_l = np.full(nwin, cap, np.int64)
    rem_h = np.full(nwin, cap, np.int64)
    slots = np.full(nwin, 64, np.int64)
    assign = np.empty(len(nodes), np.int64)
    order = np.argsort(-(deg_l[nodes] + deg_h[nodes]), kind="stable")
    for i in order:
        n = nodes[i]
        dl, dh = deg_l[n], deg_h[n]
        ok = (slots > 0) & (rem_l >= dl) & (rem_h >= dh)
        if not ok.any():
            return None
        score = np.where(ok, np.minimum(rem_l - dl, rem_h - dh), -1)
        w = int(np.argmax(score))
        assign[i] = w
        rem_l[w] -= dl
        rem_h[w] -= dh
        slots[w] -= 1
    return assign


def plan(cfg, edge_index):
    """Host planning. Returns dict with relabeling and per-core streams."""
    n, ncores = cfg.N, cfg.NCORES
    src = np.asarray(edge_index[0], np.int64)
    dst = np.asarray(edge_index[1], np.int64)
    loops = np.arange(n, dtype=np.int64)
    src_all = np.concatenate([src, loops])
    dst_all = np.concatenate([dst, loops])

    core_of = src_all // cfg.npc_real
    is_high = core_of >= (ncores // 2)
    deg_l = np.bincount(dst_all[~is_high], minlength=n)
    deg_h = np.bincount(dst_all[is_high], minlength=n)

    if cfg.S is None:
        mean = (len(src_all) / (ncores * cfg.BPC * 2 * 2))
        cfg.S = max(1, int(np.ceil(mean * 1.18 / P)))
    while True:
        cap = cfg.S * P
        assigns = []
        ok = True
        for c in range(ncores):
            nodes = np.arange(c * cfg.npc_real, (c + 1) * cfg.npc_real)
            a = _pack_core(deg_l, deg_h, nodes, cfg.BPC, cap)
            if a is None:
                ok = False
                break
            assigns.append(a)
        if ok:
            break
        cfg.S += 1

    S = cfg.S
    perm = np.empty(n, np.int64)  # orig -> new
    for c in range(ncores):
        nodes = np.arange(c * cfg.npc_real, (c + 1) * cfg.npc_real)
        a = assigns[c]
        used = np.zeros(cfg.BPC * 2, np.int64)
        for i, nd in enumerate(nodes):
            w = a[i]
            s = used[w]
            used[w] += 1
            perm[nd] = c * cfg.NPC + (w // 2) * P + (w % 2) * 64 + s
    src_new = perm[src_all]
    dst_new = perm[dst_all]

    e_core = dst_new // cfg.NPC
    e_local = dst_new % cfg.NPC
    e_blk = e_local // P
    e_win = (e_local % P) // 64
    e_slot = e_local % 64
    e_bslot = e_local % P            # block-wide slot 0..127
    e_half = (src_new >= cfg.HALFN).astype(np.int64)

    nsc_blk = 4 * S
    nsc_core = cfg.BPC * nsc_blk
    key = ((e_core * cfg.BPC + e_blk) * 2 + e_win) * 2 + e_half
    order = np.argsort(key, kind="stable")
    sorted_e = order
    key_sorted = key[order]
    nbuck = ncores * cfg.BPC * 2 * 2
    counts = np.bincount(key_sorted, minlength=nbuck)
    starts = np.concatenate([[0], np.cumsum(counts)])
    assert counts.max() <= S * P, f"bucket overflow {counts.max()} > {S*P}"

    gidx = np.zeros((ncores, nsc_core, P), np.int64)       # table row (half)
    dstl = np.full((ncores, nsc_core, P), -1.0, np.float32)  # window slot
    dtv = np.full((ncores, nsc_core, P), -1.0, np.float32)   # block slot
    for c in range(ncores):
        for b in range(cfg.BPC):
            for h in range(2):
                for w in range(2):
                    bucket = ((c * cfg.BPC + b) * 2 + w) * 2 + h
                    lo, hi = starts[bucket], starts[bucket + 1]
                    ee = sorted_e[lo:hi]
                    sc0 = b * nsc_blk + h * 2 * S + w * S
                    k = np.arange(hi - lo)
                    scs = sc0 + k // P
                    lanes = k % P
                    gi = src_new[ee] - (cfg.HALFN if h else 0)
                    gidx[c, scs, lanes] = gi
                    dstl[c, scs, lanes] = e_slot[ee]
                    dtv[c, scs, lanes] = e_bslot[ee]
    return dict(cfg=cfg, perm=perm, gidx=gidx, dstl=dstl, dtv=dtv,
                src_all=src_all, dst_all=dst_all)


def _wrap_idx(vals):
    """vals [NI] int -> wrapped [16, NI/16] int16."""
    ni = len(vals)
    assert ni % 128 == 0
    w = np.zeros((16, ni // 16), np.int16)
    w[np.arange(ni) % 16, np.arange(ni) // 16] = vals.astype(np.int16)
    return w


def make_streams(pl):
    """Per-core input arrays for the device program."""
    cfg = pl["cfg"]
    S, BPC = cfg.S, cfg.BPC
    nsc_blk = 4 * S
    out = []
    for c in range(cfg.NCORES):
        gidx, dstl, dtv = pl["gidx"][c], pl["dstl"][c], pl["dtv"][c]
        idxL_b, idxH_b = [], []
        b0 = 0
        for nb in cfg.batches:
            scs = np.arange(b0 * nsc_blk, (b0 + nb) * nsc_blk)
            b0 += nb
            blk = scs.reshape(nb, 4 * S)
            l_scs = blk[:, :2 * S].ravel()
            h_scs = blk[:, 2 * S:].ravel()
            idxL_b.append(_wrap_idx(gidx[l_scs].ravel()))
            idxH_b.append(_wrap_idx(gidx[h_scs].ravel()))
        out.append(dict(
            idxL=np.concatenate(idxL_b, axis=1),
            idxH=np.concatenate(idxH_b, axis=1),
            dstl=np.ascontiguousarray(dstl.T).astype(np.int8),  # [128, nsc]
            dstlT=dtv.ravel()[None, :].astype(np.int8),  # [1, nsc*128]
        ))
    return out


def blob_layout(cfg):
    """f16-element offsets of each section in the per-core input blob."""
    nsc_core = cfg.BPC * 4 * cfg.S
    nsc_l = nsc_core // 2
    W01 = cfg.HID + cfg.HEADS
    W2C = cfg.OUT_CH + 1
    off = {}
    o = 0
    for name, sz in (("xP", P * cfg.NPC // 2), ("Wc0", P * W01),
                     ("Wc1", P * W01),
                     ("Wc2", P * W2C), ("aW", 3 * P),
                     ("idxL", 16 * nsc_l * 8), ("idxH", 16 * nsc_l * 8),
                     ("dstl", P * nsc_core // 2),
                     ("dstlT", nsc_core * P // 2)):
        off[name] = o
        o += sz
    off["TOT"] = o
    return off


def pack_x8(xcT, s_vec):
    """[P, NPC] f32 channel-major -> per-channel int8, offset-binary u8."""
    q = np.clip(np.round(xcT / s_vec[:, None]), -128, 127) + 128
    return q.astype(np.uint8)


def fold_weights(W, a_src, a_dst, bn_g=None, bn_b=None, bn_m=None, bn_v=None,
                 bias=None):
    """Build Wc [in, hw+na] (h_tilde | adst cols), aW [128] (a_src/bn_scale,
    zero-padded), and shift tau [hw]."""
    W = np.asarray(W, np.float64)
    heads, ch = np.asarray(a_src).shape
    out_ch = W.shape[0]
    if bn_g is not None:
        s = np.asarray(bn_g, np.float64) / np.sqrt(
            np.asarray(bn_v, np.float64) + BN_EPS)
        t = np.asarray(bn_b, np.float64) - np.asarray(bn_m, np.float64) * s
    else:
        s = np.ones(out_ch)
        t = np.zeros(out_ch)
    tau = (np.asarray(bias, np.float64) * s + t) if bias is not None else t
    Wt = W.T * s[None, :]                     # [in, out] scaled
    adst_col = np.zeros((W.shape[1], heads))
    for h in range(heads):
        adst_col[:, h] = W[h * ch:(h + 1) * ch, :].T @ \
            np.asarray(a_dst, np.float64)[h]
    Wc = np.concatenate([Wt, adst_col], axis=1).astype(np.float32)
    aW = np.zeros(P, np.float32)
    aW[:out_ch] = (np.asarray(a_src, np.float64).ravel() / s).astype(
        np.float32)
    return Wc, aW, tau.astype(np.float32)


# ---------------------------------------------------------------------------
# device program
# ---------------------------------------------------------------------------

def build_program(cfg, with_tau=(False, False, False)):
    S, BPC, NCORES = cfg.S, cfg.BPC, cfg.NCORES
    HEADS, CH, OUT = cfg.HEADS, cfg.CH, cfg.OUT_CH
    HID = cfg.HID
    NPC, NPAD, HALFN = cfg.NPC, cfg.NPAD, cfg.HALFN
    nsc_blk = 4 * S
    nsc_core = BPC * nsc_blk
    W01 = HID + HEADS       # 136
    W2C = OUT + 1           # 41

    nc = bacc.Bacc("TRN2", target_bir_lowering=False, debug=False,
                   num_devices=NCORES)

    # ---- inputs (one packed blob; see blob_layout) ----
    nsc_l = nsc_core // 2
    OFF = blob_layout(cfg)
    blob = nc.dram_tensor("blob", [1, OFF["TOT"]], f16, kind="ExternalInput")
    taus = []
    for li in range(3):
        if with_tau[li]:
            w = HID if li < 2 else OUT
            taus.append(nc.dram_tensor(f"tau{li}", [1, w], f32,
                                       kind="ExternalInput"))
        else:
            taus.append(None)

    # int8 rows [q(40) | f16 scale bitcast into cols 40:42]
    yq = nc.dram_tensor("yq", [NPC, OUT + 2], i8, kind="ExternalOutput")

    # ---- internal DRAM ----
    tbl_slice = [nc.dram_tensor(f"tbs{i}", [NPC, EL], f16) for i in range(3)]
    tbl_full = [nc.dram_tensor(f"tbf{i}", [NPAD, EL], f16) for i in range(3)]
    xbuf = [nc.dram_tensor(f"xb{i}", [NPC, HID], f16) for i in range(2)]

    with tile.TileContext(nc) as tc:
        import contextlib
        ctx = contextlib.ExitStack()
        with ctx:
            const = ctx.enter_context(tc.tile_pool(name="const", bufs=1))
            nodep = ctx.enter_context(tc.tile_pool(name="nodep", bufs=2))
            npsum = ctx.enter_context(
                tc.tile_pool(name="npsum", bufs=2, space="PSUM"))
            gath = ctx.enter_context(tc.tile_pool(name="gath", bufs=2))
            blkp = ctx.enter_context(tc.tile_pool(name="blkp", bufs=2))
            apsum = ctx.enter_context(
                tc.tile_pool(name="apsum", bufs=2, space="PSUM"))

            # constants
            iota64 = const.tile([P, 64], f16)
            nc.gpsimd.iota(iota64[:], pattern=[[1, 64]], base=0,
                           channel_multiplier=0,
                           allow_small_or_imprecise_dtypes=True)
            iota128 = const.tile([P, P], f16)
            nc.gpsimd.iota(iota128[:], pattern=[[1, 128]], base=0,
                           channel_multiplier=0,
                           allow_small_or_imprecise_dtypes=True)
            iotaPP = const.tile([P, 1], i8)
            nc.gpsimd.iota(iotaPP[:], pattern=[[0, 1]], base=0,
                           channel_multiplier=1,
                           allow_small_or_imprecise_dtypes=True)
            ident = const.tile([P, P], f16)
            make_identity(nc, ident[:])
            wc_t = []
            for nm, wdt, dt_ in (("Wc0", W01, f16), ("Wc1", W01, f16),
                                 ("Wc2", W2C, f16)):
                w_sb = const.tile([P, wdt], dt_, tag=f"wc{nm}")
                nc.sync.dma_start(
                    w_sb[:], bass.AP(blob, OFF[nm], [[wdt, P], [1, wdt]]))
                wc_t.append(w_sb)
            aW_sb = const.tile([P, 3 * P], f16)
            nc.sync.dma_start(
                aW_sb[:], bass.AP(blob, OFF["aW"], [[0, P], [1, 3 * P]]))
            dstl8 = const.tile([P, nsc_core], i8)
            nc.sync.dma_start(
                dstl8[:], bass.AP(blob, OFF["dstl"],
                                  [[nsc_core // 2, P],
                                   [1, nsc_core // 2]]).bitcast(i8))
            dstl_sb = const.tile([P, nsc_core], f16)
            nc.vector.tensor_copy(dstl_sb[:], dstl8[:])
            # unpack 12-bit packed x -> xt_all [P, NPC] f16 (= round(x/s),
            # offset removed; the x scale s is folded into Wc0 on host)
            # unpack offset-binary u8 x -> xt_all f16 (per-channel scales are
            # folded into Wc0 on host, so xt holds round(x/s_c) exactly)
            xt_all = const.tile([P, NPC], f16, tag="xt_all")
            with tc.tile_pool(name="unpk", bufs=1) as unpk:
                xb = unpk.tile([P, NPC], u8, tag="xb")
                nc.sync.dma_start(
                    xb[:], bass.AP(blob, OFF["xP"],
                                   [[NPC // 2, P],
                                    [1, NPC // 2]]).bitcast(u8))
                nc.vector.tensor_scalar_add(xt_all[:], xb[:], -128.0)
            tau_t = []
            for li in range(3):
                if taus[li] is not None:
                    w = HID if li < 2 else OUT
                    tt = const.tile([P, w], f32, tag=f"tau{li}")
                    nc.sync.dma_start(
                        tt[:], bass.AP(taus[li], 0, [[0, P], [1, w]]))
                    tau_t.append(tt)
                else:
                    tau_t.append(None)
            # per-layer adst of local nodes (written in node phase, read in
            # edge phase; never leaves SBUF)
            adst0 = const.tile([P, BPC * HEADS], f16, tag="adst0")
            adst1 = const.tile([P, BPC * HEADS], f16, tag="adst1")
            adst2 = const.tile([P, BPC], f16, tag="adst2")
            adst_all = [adst0, adst1, adst2]

            def node_phase(layer):
                wdt = W01 if layer < 2 else W2C
                na = HEADS if layer < 2 else 1
                hw = HID if layer < 2 else OUT
                for t in range(BPC):
                    if layer == 0:
                        xt_ap = xt_all[:, t * P:(t + 1) * P]
                    else:
                        xin = nodep.tile([P, P], f16, tag="xin")
                        nc.sync.dma_start(
                            xin[:], xbuf[layer - 1][t * P:(t + 1) * P, :])
                        xtp = npsum.tile([P, P], f16, space="PSUM", tag="xtp")
                        nc.tensor.transpose(out=xtp[:], in_=xin[:],
                                            identity=ident[:])
                        xt = nodep.tile([P, P], f16, tag="xt16")
                        nc.vector.tensor_copy(xt[:], xtp[:])
                        xt_ap = xt[:]
                    hps = npsum.tile([P, wdt], f32, space="PSUM", tag="hps")
                    nc.tensor.matmul(out=hps[:], lhsT=xt_ap,
                                     rhs=wc_t[layer][:],
                                     start=True, stop=True)
                    hx16 = nodep.tile([P, EL], f16, tag="hx16")
                    nc.vector.tensor_copy(hx16[:, 0:hw], hps[:, 0:hw])
                    nc.sync.dma_start(
                        tbl_slice[layer][t * P:(t + 1) * P, :], hx16[:])
                    nc.vector.tensor_copy(
                        adst_all[layer][:, t * na:(t + 1) * na],
                        hps[:, hw:hw + na])
                if os.environ.get("K_NOCOLL", "0") != "1":
                    nc.gpsimd.collective_compute(
                        "AllGather", mybir.AluOpType.bypass,
                        replica_groups=[list(range(NCORES))],
                        ins=[tbl_slice[layer][:, :]],
                        outs=[tbl_full[layer][:, :]])

            g_chunk = int(os.environ.get("K_GCHUNK", "16"))
            g_sp = os.environ.get("K_SP", "0") == "1"

            def do_gather(out_tile, table_ap, idx_tile, n_sc, el):
                for c0 in range(0, n_sc, g_chunk):
                    cn = min(g_chunk, n_sc - c0)
                    o_ap = bass.AP(out_tile.tensor,
                                   out_tile[:].offset + c0 * el,
                                   [out_tile[:].ap[0], [el, cn], [1, el]])
                    i_ap = bass.AP(idx_tile.tensor,
                                   idx_tile[:].offset + c0 * 8,
                                   [idx_tile[:].ap[0], [1, cn * 8]])
                    nc.gpsimd.dma_gather(
                        out_ap=o_ap, in_ap=table_ap, idxs_ap=i_ap,
                        num_idxs=cn * P, num_idxs_reg=cn * P,
                        elem_size=el, single_packet=g_sp)

            def edge_phase(layer):
                na = HEADS if layer < 2 else 1
                hw = HID if layer < 2 else OUT
                chw = CH if layer < 2 else OUT
                mw = hw + na                 # macc row width per sub-chunk
                full = tbl_full[layer]
                aW_l = aW_sb[:, layer * P:(layer + 1) * P]
                scW = 2 * S
                b0 = 0
                offL = 0
                offT = 0
                for nb in cfg.batches:
                    nL = nb * scW
                    nA = nb * nsc_blk
                    iL = gath.tile([P, nL * 8], i16, tag="iL")
                    nc.sync.dma_start(
                        iL[:], bass.AP(blob, OFF["idxL"] + offL,
                                       [[0, 8], [nsc_l * 8, 16],
                                        [1, nL * 8]]).bitcast(i16))
                    iH = gath.tile([P, nL * 8], i16, tag="iH")
                    nc.sync.dma_start(
                        iH[:], bass.AP(blob, OFF["idxH"] + offL,
                                       [[0, 8], [nsc_l * 8, 16],
                                        [1, nL * 8]]).bitcast(i16))
                    lt = gath.tile([P, nL * EL], f16, tag="lt")
                    do_gather(lt, full[0:HALFN, :], iL, nL, EL)
                    ht = gath.tile([P, nL * EL], f16, tag="ht")
                    do_gather(ht, full[HALFN:NPAD, :], iH, nL, EL)
                    # block-slot values replicated to all partitions + indT
                    dtr = gath.tile([P, nA * P], i8, tag="dtr")
                    nc.sync.dma_start(
                        dtr[:], bass.AP(blob, OFF["dstlT"] + offT // 2,
                                        [[0, P],
                                         [1, nA * P // 2]]).bitcast(i8))
                    indT = gath.tile([P, nA * P], f16, tag="indT")
                    nc.vector.tensor_tensor(
                        out=indT[:],
                        in0=iotaPP[:, 0:1].to_broadcast([P, nA * P]),
                        in1=dtr[:], op=mybir.AluOpType.is_equal)

                    for bi in range(nb):
                        blk = b0 + bi
                        sc0 = blk * nsc_blk
                        # indicators
                        indf = blkp.tile([P, P], f16, tag="indf")
                        nc.vector.tensor_tensor(
                            out=indf[:], in0=iota128[:],
                            in1=dstl_sb[:, sc0:sc0 + 1].to_broadcast([P, P]),
                            op=mybir.AluOpType.is_equal)
                        ind = blkp.tile([P, nsc_blk * 64], f16, tag="ind")
                        in0 = bass.AP(iota64.tensor, iota64[:].offset,
                                      [iota64[:].ap[0], [0, nsc_blk],
                                       [1, 64]])
                        in1 = bass.AP(dstl_sb.tensor,
                                      dstl_sb[:, sc0:sc0 + 1].offset,
                                      [dstl_sb[:].ap[0], [1, nsc_blk],
                                       [0, 64]])
                        nc.vector.tensor_tensor(out=ind[:], in0=in0, in1=in1,
                                                op=mybir.AluOpType.is_equal)
                        # asrc recompute from gathered rows
                        asrc = blkp.tile([P, nsc_blk * na], f32, tag="asrc")
                        for half in range(2):
                            gt = lt if half == 0 else ht
                            jl0 = bi * scW
                            tmp = blkp.tile([P, scW * P], f16,
                                            tag=f"tmp{half}")
                            nc.vector.tensor_tensor(
                                out=tmp[:],
                                in0=bass.AP(gt.tensor,
                                            gt[:].offset + jl0 * EL,
                                            [gt[:].ap[0], [EL, scW],
                                             [1, P]]),
                                in1=bass.AP(aW_l.tensor, aW_l.offset,
                                            [aW_l.ap[0], [0, scW], [1, P]]),
                                op=mybir.AluOpType.mult)
                            o_ap = bass.AP(
                                asrc.tensor,
                                asrc[:].offset + half * scW * na,
                                [asrc[:].ap[0], [1, scW * na]])
                            if layer < 2:
                                i_ap = bass.AP(
                                    tmp.tensor, tmp[:].offset,
                                    [tmp[:].ap[0], [P, scW], [CH, HEADS],
                                     [1, CH]])
                            else:
                                i_ap = bass.AP(
                                    tmp.tensor, tmp[:].offset,
                                    [tmp[:].ap[0], [P, scW], [1, P]])
                            nc.vector.tensor_reduce(
                                o_ap, i_ap, axis=mybir.AxisListType.X,
                                op=mybir.AluOpType.add)
                        # adst via indT matmuls
                        eadst = apsum.tile([P, nsc_blk * na], f32,
                                           space="PSUM", tag="eadst")
                        for s in range(nsc_blk):
                            j = (bi * nsc_blk + s) * P
                            nc.tensor.matmul(
                                out=eadst[:, s * na:(s + 1) * na],
                                lhsT=indT[:, j:j + P],
                                rhs=adst_all[layer][:,
                                                    blk * na:(blk + 1) * na],
                                start=True, stop=True,
                                skip_group_check=True)
                        # e = leaky_relu(asrc + adst); ee = exp(e)
                        et = blkp.tile([P, nsc_blk * na], f32, tag="et")
                        nc.vector.tensor_tensor(out=et[:], in0=asrc[:],
                                                in1=eadst[:],
                                                op=mybir.AluOpType.add)
                        elr = blkp.tile([P, nsc_blk * na], f32, tag="elr")
                        nc.vector.scalar_tensor_tensor(
                            out=elr[:], in0=et[:], scalar=NEG_SLOPE,
                            in1=et[:], op0=mybir.AluOpType.mult,
                            op1=mybir.AluOpType.max)
                        ee = blkp.tile([P, nsc_blk * na], f16, tag="ee")
                        nc.scalar.activation(ee[:], elr[:],
                                             mybir.ActivationFunctionType.Exp)
                        # macc = [h * ee | ee] per sub-chunk
                        macc = blkp.tile([P, nsc_blk * mw], f16, tag="macc")
                        for half in range(2):
                            gt = lt if half == 0 else ht
                            jl0 = bi * scW
                            nc.vector.tensor_tensor(
                                out=bass.AP(
                                    macc.tensor,
                                    macc[:].offset + half * scW * mw,
                                    [macc[:].ap[0], [mw, scW], [1, hw]]),
                                in0=bass.AP(gt.tensor,
                                            gt[:].offset + jl0 * EL,
                                            [gt[:].ap[0], [EL, scW],
                                             [1, hw]]),
                                in1=bass.AP(ee.tensor,
                                            ee[:].offset + half * scW * na,
                                            [ee[:].ap[0], [1, scW * na],
                                             [0, chw]]),
                                op=mybir.AluOpType.mult)
                            nc.vector.tensor_copy(
                                bass.AP(
                                    macc.tensor,
                                    macc[:].offset + half * scW * mw + hw,
                                    [macc[:].ap[0], [mw, scW], [1, na]]),
                                bass.AP(ee.tensor,
                                        ee[:].offset + half * scW * na,
                                        [ee[:].ap[0], [na, scW], [1, na]]))
                        # accumulate [msg | den] into PSUM
                        acc = apsum.tile([P, mw], f32, space="PSUM",
                                         tag="acc")
                        for s in range(nsc_blk):
                            first = s == 0
                            last = s == nsc_blk - 1
                            if first:
                                lhs = indf[:]
                                rows = acc[:, :]
                            else:
                                w = (s % scW) // S
                                lhs = ind[:, s * 64:(s + 1) * 64]
                                rows = acc[w * 64:(w + 1) * 64, :]
                            nc.tensor.matmul(
                                out=rows[:, 0:mw],
                                lhsT=lhs,
                                rhs=macc[:, s * mw:(s + 1) * mw],
                                start=first, stop=last,
                                skip_group_check=True)
                        # finalize block
                        den = blkp.tile([P, na], f32, tag="den")
                        nc.vector.tensor_scalar_add(
                            den[:], acc[:, hw:hw + na], 1e-16)
                        rec = blkp.tile([P, na], f32, tag="rec")
                        nc.vector.reciprocal(rec[:], den[:])
                        xo = blkp.tile([P, hw], f32, tag="xo")
                        rec_b = bass.AP(rec.tensor, rec[:].offset,
                                        [rec[:].ap[0], [1, na], [0, chw]])
                        nc.vector.tensor_tensor(out=xo[:], in0=acc[:, 0:hw],
                                                in1=rec_b,
                                                op=mybir.AluOpType.mult)
                        if tau_t[layer] is not None:
                            nc.vector.tensor_tensor(
                                out=xo[:], in0=xo[:], in1=tau_t[layer][:],
                                op=mybir.AluOpType.add)
                        if layer < 2:
                            ng = blkp.tile([P, hw], f32, tag="ng")
                            nc.vector.tensor_scalar_min(ng[:], xo[:], 0.0)
                            en = blkp.tile([P, hw], f32, tag="en")
                            nc.scalar.activation(
                                en[:], ng[:],
                                mybir.ActivationFunctionType.Exp)
                            ps = blkp.tile([P, hw], f32, tag="ps")
                            nc.vector.tensor_scalar_max(ps[:], xo[:], 0.0)
                            xe = blkp.tile([P, hw], f16, tag="xe")
                            nc.vector.scalar_tensor_tensor(
                                out=xe[:], in0=en[:], scalar=-1.0,
                                in1=ps[:], op0=mybir.AluOpType.add,
                                op1=mybir.AluOpType.add)
                            nc.sync.dma_start(
                                xbuf[layer][blk * P:(blk + 1) * P, :], xe[:])
                        else:
                            # int8 quantize with per-row scale
                            rmax = blkp.tile([P, 1], f32, tag="rmax")
                            nc.vector.tensor_reduce(
                                rmax[:], xo[:], axis=mybir.AxisListType.X,
                                op=mybir.AluOpType.max,
                                apply_absolute_value=True)
                            rmx = blkp.tile([P, 1], f32, tag="rmx")
                            nc.vector.tensor_scalar_max(rmx[:], rmax[:],
                                                        1e-6)
                            rs = blkp.tile([P, 1], f32, tag="rs")
                            nc.vector.reciprocal(rs[:], rmx[:])
                            yqf = blkp.tile([P, OUT], f32, tag="yqf")
                            nc.vector.scalar_tensor_tensor(
                                out=yqf[:], in0=xo[:], scalar=127.0,
                                in1=bass.AP(rs.tensor, rs[:].offset,
                                            [rs[:].ap[0], [0, OUT]]),
                                op0=mybir.AluOpType.mult,
                                op1=mybir.AluOpType.mult)
                            yo = blkp.tile([P, OUT + 2], i8, tag="yo")
                            nc.vector.tensor_copy(yo[:, 0:OUT], yqf[:])
                            nc.vector.tensor_scalar_mul(
                                yo[:, OUT:OUT + 2].bitcast(f16), rmx[:],
                                1.0 / 127.0)
                            nc.sync.dma_start(
                                yq[blk * P:(blk + 1) * P, :], yo[:])
                    b0 += nb
                    offL += nL * 8
                    offT += nA * P

            nlayers = int(os.environ.get("K_LAYERS", "3"))
            do_edge = os.environ.get("K_EDGE", "1") == "1"
            nrep = int(os.environ.get("K_REPEAT", "1"))
            for _rep in range(nrep):
                for layer in range(nlayers):
                    node_phase(layer)
                    if do_edge:
                        edge_phase(layer)
            if nlayers < 3 or not do_edge:
                zt = blkp.tile([P, OUT + 2], i8, tag="ytouch")
                nc.gpsimd.memset(zt[:], 0.0)
                for blk in range(BPC):
                    nc.sync.dma_start(yq[blk * P:(blk + 1) * P, :], zt[:])
    nc.compile()
    return nc


# ---------------------------------------------------------------------------
# numpy mirror of the device pipeline (for plan/stream validation)
# ---------------------------------------------------------------------------

def numpy_pipeline(pl, x, Wcs, aWs, taus):
    cfg = pl["cfg"]
    perm = pl["perm"]
    xp = np.zeros((cfg.NPAD, cfg.IN_CH), np.float32)
    xp[perm] = x
    S = cfg.S
    nsc_blk = 4 * S
    for layer in range(3):
        hw = cfg.HID if layer < 2 else cfg.OUT_CH
        na = cfg.HEADS if layer < 2 else 1
        chw = cfg.CH if layer < 2 else cfg.OUT_CH
        h = (xp @ Wcs[layer]).astype(np.float32)   # [NPAD, hw+na]
        table = np.zeros((cfg.NPAD, EL), np.float16)
        table[:, 0:hw] = h[:, 0:hw].astype(np.float16)
        adst16 = h[:, hw:hw + na].astype(np.float16)
        aW = aWs[layer]
        xnew = np.zeros((cfg.NPAD, hw), np.float32)
        for c in range(cfg.NCORES):
            gidx, dstl, dtv = pl["gidx"][c], pl["dstl"][c], pl["dtv"][c]
            for blk in range(cfg.BPC):
                accw = np.zeros((P, hw + na), np.float32)
                adst_blk = adst16[c * cfg.NPC + blk * P:
                                  c * cfg.NPC + (blk + 1) * P]
                for s in range(nsc_blk):
                    sc = blk * nsc_blk + s
                    half = 0 if s < 2 * S else 1
                    w = (s % (2 * S)) // S
                    gi = gidx[sc] + (cfg.HALFN if half else 0)
                    rows = table[gi]                         # [P, EL] f16
                    # grouped reduce: heads of width hw//na (aW zero-padded)
                    asrc = (rows.astype(np.float32) * aW[None, :])[
                        :, 0:hw].reshape(P, na, hw // na).sum(-1)
                    dv = dtv[sc].astype(np.int64)
                    ad = np.where(dv[:, None] >= 0,
                                  adst_blk[np.maximum(dv, 0)].astype(
                                      np.float32), 0.0)
                    e = asrc + ad
                    e = np.maximum(e, NEG_SLOPE * e)
                    ee = np.exp(e).astype(np.float16)
                    msg = (rows[:, 0:hw] *
                           np.repeat(ee, chw, axis=1)).astype(np.float16)
                    sl = dstl[sc]
                    mrow = np.concatenate(
                        [msg.astype(np.float32), ee.astype(np.float32)], 1)
                    if s == 0:
                        indm = (np.arange(P)[None, :] == sl[:, None])
                        accw += indm.astype(np.float32).T @ mrow
                    else:
                        indm = (np.arange(64)[None, :] == sl[:, None])
                        accw[w * 64:(w + 1) * 64] += \
                            indm.astype(np.float32).T @ mrow
                rec = 1.0 / (accw[:, hw:hw + na] + 1e-16)
                xo = accw[:, 0:hw] * np.repeat(rec, chw, axis=1)
                if taus[layer] is not None:
                    xo = xo + taus[layer][None, :]
                if layer < 2:
                    xo = np.where(xo > 0, xo,
                                  np.exp(np.minimum(xo, 0)) - 1)
                xnew[c * cfg.NPC + blk * P:c * cfg.NPC + (blk + 1) * P] = \
                    xo.astype(np.float16).astype(np.float32)
        xp = xnew
    return xp[perm]


# ---------------------------------------------------------------------------
# public entry
# ---------------------------------------------------------------------------

_CACHE = {}


def _get_program(cfg_key, cfg, with_tau):
    key = (cfg_key, with_tau, cfg.S)
    if key not in _CACHE:
        _CACHE[key] = build_program(cfg, with_tau)
    return _CACHE[key]


def prep_inputs(cfg, inputs):
    """Host planning + per-core input maps. Returns (nc, in_maps, perm)."""
    x = np.asarray(inputs["x"], np.float32)
    edge_index = np.asarray(inputs["edge_index"])
    pl = plan(cfg, edge_index)
    streams = make_streams(pl)
    Wc0, aW0, tau0 = fold_weights(
        inputs["W0"], inputs["a_src0"], inputs["a_dst0"], inputs["bn_g0"],
        inputs["bn_b0"], inputs["bn_m0"], inputs["bn_v0"], inputs["b0"])
    Wc1, aW1, tau1 = fold_weights(
        inputs["W1"], inputs["a_src1"], inputs["a_dst1"], inputs["bn_g1"],
        inputs["bn_b1"], inputs["bn_m1"], inputs["bn_v1"], inputs["b1"])
    Wc2, aW2, tau2 = fold_weights(
        inputs["W2"], inputs["a_src2"], inputs["a_dst2"], bias=inputs["b2"])
    taus = [tau0, tau1, tau2]
    with_tau = tuple(bool(np.any(t != 0)) for t in taus)
    nc = _get_program("gat", cfg, with_tau)

    perm = pl["perm"]
    OFF = blob_layout(cfg)
    aW_row = np.concatenate([aW0, aW1, aW2]).astype(np.float16)
    xs = np.maximum(np.abs(x).max(axis=0), 1e-9) / 127.0  # per input channel
    wc16 = [(Wc0 * xs[:, None]).astype(np.float16).ravel(),
            Wc1.astype(np.float16).ravel(),
            Wc2.astype(np.float16).ravel()]
    in_maps = []
    for c in range(cfg.NCORES):
        xc = np.zeros((cfg.NPC, cfg.IN_CH), np.float32)
        mask = (perm >= c * cfg.NPC) & (perm < (c + 1) * cfg.NPC)
        origs = np.nonzero(mask)[0]
        xc[perm[origs] - c * cfg.NPC] = x[origs]
        xcT = np.ascontiguousarray(xc.T)
        s = streams[c]
        blob = np.concatenate([
            pack_x8(xcT, xs).ravel().view(np.float16),
            wc16[0], wc16[1], wc16[2], aW_row,
            s["idxL"].ravel().view(np.float16),
            s["idxH"].ravel().view(np.float16),
            s["dstl"].ravel().view(np.float16),
            s["dstlT"].ravel().view(np.float16),
        ])[None, :]
        assert blob.shape[1] == OFF["TOT"], (blob.shape, OFF["TOT"])
        m = dict(blob=blob)
        for li, t in enumerate(taus):
            if with_tau[li]:
                m[f"tau{li}"] = t[None, :].astype(np.float32)
        in_maps.append(m)
    return nc, in_maps, perm, (pl, [Wc0, Wc1, Wc2], [aW0, aW1, aW2], taus,
                               with_tau)


def unquant_y(results, ncores):
    """[core]{yq} (int8 rows [q(40)|f16 scale]) -> full [NPAD, OUT] f32."""
    outs = []
    for c in range(ncores):
        raw = np.ascontiguousarray(results[c]["yq"])
        nout = raw.shape[1] - 2
        q = raw[:, 0:nout].astype(np.float32)
        s = raw[:, nout:nout + 2].copy().view(np.float16).astype(np.float32)
        outs.append(q * s)
    return np.concatenate(outs, axis=0)


def run(cfg, inputs, trace=False):
    nc, in_maps, perm, _ = prep_inputs(cfg, inputs)
    res = run_bass_kernel_spmd(nc, in_maps, list(range(cfg.NCORES)),
                               trace=trace)
    out = unquant_y(res.results, cfg.NCORES)[perm]
    return out, res


def kernel(**inputs):
    cfg = Cfg(n=50000, e=800000)
    out, _ = run(cfg, inputs)
    return out.astype(np.float32)


# revision 53
# speedup vs baseline: 1.1684x; 1.1684x over previous
"""GAT (3-layer) Trainium2 Bass kernel, 8-core SPMD.

Schedule:
 - Nodes are relabeled and packed by the host into a uniform schedule:
   8 cores x B blocks x 2 windows x 64 slots. Each (window, src-half) gets a
   fixed number S of 128-edge sub-chunks; all per-core variation lives in
   data (gather indices / dst-slot metadata), so one SPMD program serves all
   cores. Per layer: node phase (h = x @ Wc, AllGather of node tables),
   then edge phase (bulk dma_gather of h[src] rows, segment softmax +
   weighted sums via indicator matmuls accumulating in PSUM).

Data-plane design (end-to-end I/O is the dominant cost):
 - Node tables are [NPAD, 128] f16 (256B rows == minimum dma_gather
   granularity): row = h_tilde = (x @ W.T) * bn_scale (BN folded on host).
 - asrc is recomputed per edge on-chip: grouped reduce over the gathered
   h_tilde row against aW = a_src/bn_scale. No asrc in the table rows.
 - adst never hits DRAM: per block the 128 local-node adst values (kept in
   SBUF from the node phase) are spread to edge lanes with an
   indicator-transpose matmul, indT = is_equal(iota_p, dstlT).
 - Per sub-chunk ONE accumulation matmul: rhs = [h*ee | ee] so messages and
   softmax denominators accumulate together in PSUM.
 - All inputs ship as ONE packed blob per core (~1.4 MB): x quantized to
   int8 with per-input-channel scales folded into Wc0 (one DMA + one DVE
   add to unpack), de-replicated [16, W] int16 index streams (tiled to
   [128, W] by DMA), int8 dstl/dstlT, f16 weights.
 - Output is ONE int8 array per core: rows [q(40) | per-row f16 scale
   bitcast into the last two bytes], dequantized on host.
"""
import os
import numpy as np

os.environ.setdefault("JAX_COMPILATION_CACHE_DIR", "/tmp/jaxcache")

import concourse.bass as bass
import concourse.bacc as bacc
import concourse.tile as tile
import concourse.mybir as mybir
from concourse.bass_utils import run_bass_kernel_spmd
from concourse.masks import make_identity

P = 128
f32 = mybir.dt.float32
f16 = mybir.dt.float16
i16 = mybir.dt.int16
i8 = mybir.dt.int8
u8 = mybir.dt.uint8
u16 = mybir.dt.uint16

NEG_SLOPE = 0.2
BN_EPS = 1e-5
EL = 128            # f16 elems per table row (256B = min gather granularity)


class Cfg:
    def __init__(self, n, e, ncores=8, bpc=None, s=None, batch_blocks=4,
                 heads=8, ch=16, out_ch=40, in_ch=128):
        self.N = n
        self.E = e
        self.NCORES = ncores
        self.IN_CH = in_ch
        self.HID = heads * ch
        self.HEADS = heads
        self.CH = ch
        self.OUT_CH = out_ch
        assert n % ncores == 0
        self.npc_real = n // ncores
        self.BPC = bpc if bpc else (self.npc_real + P - 1) // P
        self.NPC = self.BPC * P              # node slots per core
        self.NPAD = self.NPC * ncores        # total node slots
        self.HALFN = self.NPAD // 2          # table half size
        assert ncores % 2 == 0
        assert self.HALFN < 32768, "half table must be int16 indexable"
        self.S = s
        bb = []
        nb = self.BPC
        while nb > 0:
            take = min(batch_blocks, nb)
            bb.append(take)
            nb -= take
        self.batches = bb


def _pack_core(deg_l, deg_h, nodes, bpc, cap):
    """Greedy 2D bin packing: nodes (orig ids) -> window. bins = bpc*2
    windows with 64 slots, capacity cap on both L and H edge sums."""
    nwin = bpc * 2
    rem_l = np.full(nwin, cap, np.int64)
    rem_h = np.full(nwin, cap, np.int64)
    slots = np.full(nwin, 64, np.int64)
    assign = np.empty(len(nodes), np.int64)
    order = np.argsort(-(deg_l[nodes] + deg_h[nodes]), kind="stable")
    for i in order:
        n = nodes[i]
        dl, dh = deg_l[n], deg_h[n]
        ok = (slots > 0) & (rem_l >= dl) & (rem_h >= dh)
        if not ok.any():
            return None
        score = np.where(ok, np.minimum(rem_l - dl, rem_h - dh), -1)
        w = int(np.argmax(score))
        assign[i] = w
        rem_l[w] -= dl
        rem_h[w] -= dh
        slots[w] -= 1
    return assign


def plan(cfg, edge_index):
    """Host planning. Returns dict with relabeling and per-core streams."""
    n, ncores = cfg.N, cfg.NCORES
    src = np.asarray(edge_index[0], np.int64)
    dst = np.asarray(edge_index[1], np.int64)
    loops = np.arange(n, dtype=np.int64)
    src_all = np.concatenate([src, loops])
    dst_all = np.concatenate([dst, loops])

    core_of = src_all // cfg.npc_real
    is_high = core_of >= (ncores // 2)
    deg_l = np.bincount(dst_all[~is_high], minlength=n)
    deg_h = np.bincount(dst_all[is_high], minlength=n)

    if cfg.S is None:
        mean = (len(src_all) / (ncores * cfg.BPC * 2 * 2))
        cfg.S = max(1, int(np.ceil(mean * 1.18 / P)))
    while True:
        cap = cfg.S * P
        assigns = []
        ok = True
        for c in range(ncores):
            nodes = np.arange(c * cfg.npc_real, (c + 1) * cfg.npc_real)
            a = _pack_core(deg_l, deg_h, nodes, cfg.BPC, cap)
            if a is None:
                ok = False
                break
            assigns.append(a)
        if ok:
            break
        cfg.S += 1

    S = cfg.S
    perm = np.empty(n, np.int64)  # orig -> new
    for c in range(ncores):
        nodes = np.arange(c * cfg.npc_real, (c + 1) * cfg.npc_real)
        a = assigns[c]
        used = np.zeros(cfg.BPC * 2, np.int64)
        for i, nd in enumerate(nodes):
            w = a[i]
            s = used[w]
            used[w] += 1
            perm[nd] = c * cfg.NPC + (w // 2) * P + (w % 2) * 64 + s
    src_new = perm[src_all]
    dst_new = perm[dst_all]

    e_core = dst_new // cfg.NPC
    e_local = dst_new % cfg.NPC
    e_blk = e_local // P
    e_win = (e_local % P) // 64
    e_slot = e_local % 64
    e_bslot = e_local % P            # block-wide slot 0..127
    e_half = (src_new >= cfg.HALFN).astype(np.int64)

    nsc_blk = 4 * S
    nsc_core = cfg.BPC * nsc_blk
    key = ((e_core * cfg.BPC + e_blk) * 2 + e_win) * 2 + e_half
    order = np.argsort(key, kind="stable")
    sorted_e = order
    key_sorted = key[order]
    nbuck = ncores * cfg.BPC * 2 * 2
    counts = np.bincount(key_sorted, minlength=nbuck)
    starts = np.concatenate([[0], np.cumsum(counts)])
    assert counts.max() <= S * P, f"bucket overflow {counts.max()} > {S*P}"

    gidx = np.zeros((ncores, nsc_core, P), np.int64)       # table row (half)
    dstl = np.full((ncores, nsc_core, P), -1.0, np.float32)  # window slot
    dtv = np.full((ncores, nsc_core, P), -1.0, np.float32)   # block slot
    for c in range(ncores):
        for b in range(cfg.BPC):
            for h in range(2):
                for w in range(2):
                    bucket = ((c * cfg.BPC + b) * 2 + w) * 2 + h
                    lo, hi = starts[bucket], starts[bucket + 1]
                    ee = sorted_e[lo:hi]
                    sc0 = b * nsc_blk + h * 2 * S + w * S
                    k = np.arange(hi - lo)
                    scs = sc0 + k // P
                    lanes = k % P
                    gi = src_new[ee] - (cfg.HALFN if h else 0)
                    gidx[c, scs, lanes] = gi
                    dstl[c, scs, lanes] = e_slot[ee]
                    dtv[c, scs, lanes] = e_bslot[ee]
    return dict(cfg=cfg, perm=perm, gidx=gidx, dstl=dstl, dtv=dtv,
                src_all=src_all, dst_all=dst_all)


def _wrap_idx(vals):
    """vals [NI] int -> wrapped [16, NI/16] int16."""
    ni = len(vals)
    assert ni % 128 == 0
    w = np.zeros((16, ni // 16), np.int16)
    w[np.arange(ni) % 16, np.arange(ni) // 16] = vals.astype(np.int16)
    return w


def make_streams(pl):
    """Per-core input arrays for the device program."""
    cfg = pl["cfg"]
    S, BPC = cfg.S, cfg.BPC
    nsc_blk = 4 * S
    out = []
    for c in range(cfg.NCORES):
        gidx, dstl, dtv = pl["gidx"][c], pl["dstl"][c], pl["dtv"][c]
        idxL_b, idxH_b = [], []
        b0 = 0
        for nb in cfg.batches:
            scs = np.arange(b0 * nsc_blk, (b0 + nb) * nsc_blk)
            b0 += nb
            blk = scs.reshape(nb, 4 * S)
            l_scs = blk[:, :2 * S].ravel()
            h_scs = blk[:, 2 * S:].ravel()
            idxL_b.append(_wrap_idx(gidx[l_scs].ravel()))
            idxH_b.append(_wrap_idx(gidx[h_scs].ravel()))
        out.append(dict(
            idxL=np.concatenate(idxL_b, axis=1),
            idxH=np.concatenate(idxH_b, axis=1),
            dstl=np.ascontiguousarray(dstl.T).astype(np.int8),  # [128, nsc]
            dstlT=dtv.ravel()[None, :].astype(np.int8),  # [1, nsc*128]
        ))
    return out


def blob_layout(cfg):
    """f16-element offsets of each section in the per-core input blob."""
    nsc_core = cfg.BPC * 4 * cfg.S
    nsc_l = nsc_core // 2
    W01 = cfg.HID + cfg.HEADS
    W2C = cfg.OUT_CH + 1
    off = {}
    o = 0
    for name, sz in (("xP", P * cfg.NPC // 2), ("Wc0", P * W01),
                     ("Wc1", P * W01),
                     ("Wc2", P * W2C), ("aW", 3 * P),
                     ("idxL", 16 * nsc_l * 8), ("idxH", 16 * nsc_l * 8),
                     ("dstl", P * nsc_core // 2),
                     ("dstlT", nsc_core * P // 2)):
        off[name] = o
        o += sz
    off["TOT"] = o
    return off


def pack_x8(xcT, s_vec):
    """[P, NPC] f32 channel-major -> per-channel int8, offset-binary u8."""
    q = np.clip(np.round(xcT / s_vec[:, None]), -128, 127) + 128
    return q.astype(np.uint8)


def fold_weights(W, a_src, a_dst, bn_g=None, bn_b=None, bn_m=None, bn_v=None,
                 bias=None):
    """Build Wc [in, hw+na] (h_tilde | adst cols), aW [128] (a_src/bn_scale,
    zero-padded), and shift tau [hw]."""
    W = np.asarray(W, np.float64)
    heads, ch = np.asarray(a_src).shape
    out_ch = W.shape[0]
    if bn_g is not None:
        s = np.asarray(bn_g, np.float64) / np.sqrt(
            np.asarray(bn_v, np.float64) + BN_EPS)
        t = np.asarray(bn_b, np.float64) - np.asarray(bn_m, np.float64) * s
    else:
        s = np.ones(out_ch)
        t = np.zeros(out_ch)
    tau = (np.asarray(bias, np.float64) * s + t) if bias is not None else t
    Wt = W.T * s[None, :]                     # [in, out] scaled
    adst_col = np.zeros((W.shape[1], heads))
    for h in range(heads):
        adst_col[:, h] = W[h * ch:(h + 1) * ch, :].T @ \
            np.asarray(a_dst, np.float64)[h]
    Wc = np.concatenate([Wt, adst_col], axis=1).astype(np.float32)
    aW = np.zeros(P, np.float32)
    aW[:out_ch] = (np.asarray(a_src, np.float64).ravel() / s).astype(
        np.float32)
    return Wc, aW, tau.astype(np.float32)


# ---------------------------------------------------------------------------
# device program
# ---------------------------------------------------------------------------

def build_program(cfg, with_tau=(False, False, False)):
    S, BPC, NCORES = cfg.S, cfg.BPC, cfg.NCORES
    HEADS, CH, OUT = cfg.HEADS, cfg.CH, cfg.OUT_CH
    HID = cfg.HID
    NPC, NPAD, HALFN = cfg.NPC, cfg.NPAD, cfg.HALFN
    nsc_blk = 4 * S
    nsc_core = BPC * nsc_blk
    W01 = HID + HEADS       # 136
    W2C = OUT + 1           # 41

    nc = bacc.Bacc("TRN2", target_bir_lowering=False, debug=False,
                   num_devices=NCORES)

    # ---- inputs (one packed blob; see blob_layout) ----
    nsc_l = nsc_core // 2
    OFF = blob_layout(cfg)
    blob = nc.dram_tensor("blob", [1, OFF["TOT"]], f16, kind="ExternalInput")
    taus = []
    for li in range(3):
        if with_tau[li]:
            w = HID if li < 2 else OUT
            taus.append(nc.dram_tensor(f"tau{li}", [1, w], f32,
                                       kind="ExternalInput"))
        else:
            taus.append(None)

    # int8 rows [q(40) | f16 scale bitcast into cols 40:42]
    yq = nc.dram_tensor("yq", [NPC, OUT + 2], i8, kind="ExternalOutput")

    # ---- internal DRAM ----
    tbl_slice = [nc.dram_tensor(f"tbs{i}", [NPC, EL], f16) for i in range(3)]
    tbl_full = [nc.dram_tensor(f"tbf{i}", [NPAD, EL], f16) for i in range(3)]
    xbuf = [nc.dram_tensor(f"xb{i}", [NPC, HID], f16) for i in range(2)]

    with tile.TileContext(nc) as tc:
        import contextlib
        ctx = contextlib.ExitStack()
        with ctx:
            const = ctx.enter_context(tc.tile_pool(name="const", bufs=1))
            nodep = ctx.enter_context(tc.tile_pool(name="nodep", bufs=2))
            npsum = ctx.enter_context(
                tc.tile_pool(name="npsum", bufs=2, space="PSUM"))
            gath = ctx.enter_context(tc.tile_pool(name="gath", bufs=2))
            blkp = ctx.enter_context(tc.tile_pool(name="blkp", bufs=2))
            apsum = ctx.enter_context(
                tc.tile_pool(name="apsum", bufs=2, space="PSUM"))

            # constants
            iota64 = const.tile([P, 64], f16)
            nc.gpsimd.iota(iota64[:], pattern=[[1, 64]], base=0,
                           channel_multiplier=0,
                           allow_small_or_imprecise_dtypes=True)
            iota128 = const.tile([P, P], f16)
            nc.gpsimd.iota(iota128[:], pattern=[[1, 128]], base=0,
                           channel_multiplier=0,
                           allow_small_or_imprecise_dtypes=True)
            iotaPP = const.tile([P, 1], i8)
            nc.gpsimd.iota(iotaPP[:], pattern=[[0, 1]], base=0,
                           channel_multiplier=1,
                           allow_small_or_imprecise_dtypes=True)
            ident = const.tile([P, P], f16)
            make_identity(nc, ident[:])
            wc_t = []
            for nm, wdt, dt_ in (("Wc0", W01, f16), ("Wc1", W01, f16),
                                 ("Wc2", W2C, f16)):
                w_sb = const.tile([P, wdt], dt_, tag=f"wc{nm}")
                nc.sync.dma_start(
                    w_sb[:], bass.AP(blob, OFF[nm], [[wdt, P], [1, wdt]]))
                wc_t.append(w_sb)
            aW_sb = const.tile([P, 3 * P], f16)
            nc.sync.dma_start(
                aW_sb[:], bass.AP(blob, OFF["aW"], [[0, P], [1, 3 * P]]))
            dstl8 = const.tile([P, nsc_core], i8)
            nc.sync.dma_start(
                dstl8[:], bass.AP(blob, OFF["dstl"],
                                  [[nsc_core // 2, P],
                                   [1, nsc_core // 2]]).bitcast(i8))
            dstl_sb = const.tile([P, nsc_core], f16)
            nc.vector.tensor_copy(dstl_sb[:], dstl8[:])
            # unpack 12-bit packed x -> xt_all [P, NPC] f16 (= round(x/s),
            # offset removed; the x scale s is folded into Wc0 on host)
            # unpack offset-binary u8 x -> xt_all f16 (per-channel scales are
            # folded into Wc0 on host, so xt holds round(x/s_c) exactly)
            xt_all = const.tile([P, NPC], f16, tag="xt_all")
            with tc.tile_pool(name="unpk", bufs=1) as unpk:
                xb = unpk.tile([P, NPC], u8, tag="xb")
                nc.sync.dma_start(
                    xb[:], bass.AP(blob, OFF["xP"],
                                   [[NPC // 2, P],
                                    [1, NPC // 2]]).bitcast(u8))
                nc.vector.tensor_scalar_add(xt_all[:], xb[:], -128.0)
            tau_t = []
            for li in range(3):
                if taus[li] is not None:
                    w = HID if li < 2 else OUT
                    tt = const.tile([P, w], f32, tag=f"tau{li}")
                    nc.sync.dma_start(
                        tt[:], bass.AP(taus[li], 0, [[0, P], [1, w]]))
                    tau_t.append(tt)
                else:
                    tau_t.append(None)
            # per-layer adst of local nodes (written in node phase, read in
            # edge phase; never leaves SBUF)
            adst0 = const.tile([P, BPC * HEADS], f16, tag="adst0")
            adst1 = const.tile([P, BPC * HEADS], f16, tag="adst1")
            adst2 = const.tile([P, BPC], f16, tag="adst2")
            adst_all = [adst0, adst1, adst2]

            def node_phase(layer):
                wdt = W01 if layer < 2 else W2C
                na = HEADS if layer < 2 else 1
                hw = HID if layer < 2 else OUT
                for t in range(BPC):
                    if layer == 0:
                        xt_ap = xt_all[:, t * P:(t + 1) * P]
                    else:
                        xin = nodep.tile([P, P], f16, tag="xin")
                        nc.sync.dma_start(
                            xin[:], xbuf[layer - 1][t * P:(t + 1) * P, :])
                        xtp = npsum.tile([P, P], f16, space="PSUM", tag="xtp")
                        nc.tensor.transpose(out=xtp[:], in_=xin[:],
                                            identity=ident[:])
                        xt = nodep.tile([P, P], f16, tag="xt16")
                        nc.vector.tensor_copy(xt[:], xtp[:])
                        xt_ap = xt[:]
                    hps = npsum.tile([P, wdt], f32, space="PSUM", tag="hps")
                    nc.tensor.matmul(out=hps[:], lhsT=xt_ap,
                                     rhs=wc_t[layer][:],
                                     start=True, stop=True)
                    hx16 = nodep.tile([P, EL], f16, tag="hx16")
                    nc.vector.tensor_copy(hx16[:, 0:hw], hps[:, 0:hw])
                    nc.sync.dma_start(
                        tbl_slice[layer][t * P:(t + 1) * P, :], hx16[:])
                    nc.vector.tensor_copy(
                        adst_all[layer][:, t * na:(t + 1) * na],
                        hps[:, hw:hw + na])
                if os.environ.get("K_NOCOLL", "0") != "1":
                    nc.gpsimd.collective_compute(
                        "AllGather", mybir.AluOpType.bypass,
                        replica_groups=[list(range(NCORES))],
                        ins=[tbl_slice[layer][:, :]],
                        outs=[tbl_full[layer][:, :]])

            g_chunk = int(os.environ.get("K_GCHUNK", "16"))
            g_sp = os.environ.get("K_SP", "0") == "1"

            def do_gather(out_tile, table_ap, idx_tile, n_sc, el):
                for c0 in range(0, n_sc, g_chunk):
                    cn = min(g_chunk, n_sc - c0)
                    o_ap = bass.AP(out_tile.tensor,
                                   out_tile[:].offset + c0 * el,
                                   [out_tile[:].ap[0], [el, cn], [1, el]])
                    i_ap = bass.AP(idx_tile.tensor,
                                   idx_tile[:].offset + c0 * 8,
                                   [idx_tile[:].ap[0], [1, cn * 8]])
                    nc.gpsimd.dma_gather(
                        out_ap=o_ap, in_ap=table_ap, idxs_ap=i_ap,
                        num_idxs=cn * P, num_idxs_reg=cn * P,
                        elem_size=el, single_packet=g_sp)

            def edge_phase(layer):
                na = HEADS if layer < 2 else 1
                hw = HID if layer < 2 else OUT
                chw = CH if layer < 2 else OUT
                mw = hw + na                 # macc row width per sub-chunk
                full = tbl_full[layer]
                aW_l = aW_sb[:, layer * P:(layer + 1) * P]
                scW = 2 * S
                b0 = 0
                offL = 0
                offT = 0
                for nb in cfg.batches:
                    nL = nb * scW
                    nA = nb * nsc_blk
                    iL = gath.tile([P, nL * 8], i16, tag="iL")
                    nc.sync.dma_start(
                        iL[:], bass.AP(blob, OFF["idxL"] + offL,
                                       [[0, 8], [nsc_l * 8, 16],
                                        [1, nL * 8]]).bitcast(i16))
                    iH = gath.tile([P, nL * 8], i16, tag="iH")
                    nc.sync.dma_start(
                        iH[:], bass.AP(blob, OFF["idxH"] + offL,
                                       [[0, 8], [nsc_l * 8, 16],
                                        [1, nL * 8]]).bitcast(i16))
                    lt = gath.tile([P, nL * EL], f16, tag="lt")
                    do_gather(lt, full[0:HALFN, :], iL, nL, EL)
                    ht = gath.tile([P, nL * EL], f16, tag="ht")
                    do_gather(ht, full[HALFN:NPAD, :], iH, nL, EL)
                    # block-slot values replicated to all partitions + indT
                    dtr = gath.tile([P, nA * P], i8, tag="dtr")
                    nc.sync.dma_start(
                        dtr[:], bass.AP(blob, OFF["dstlT"] + offT // 2,
                                        [[0, P],
                                         [1, nA * P // 2]]).bitcast(i8))
                    indT = gath.tile([P, nA * P], f16, tag="indT")
                    nc.vector.tensor_tensor(
                        out=indT[:],
                        in0=iotaPP[:, 0:1].to_broadcast([P, nA * P]),
                        in1=dtr[:], op=mybir.AluOpType.is_equal)

                    for bi in range(nb):
                        blk = b0 + bi
                        sc0 = blk * nsc_blk
                        # indicators
                        indf = blkp.tile([P, P], f16, tag="indf")
                        nc.vector.tensor_tensor(
                            out=indf[:], in0=iota128[:],
                            in1=dstl_sb[:, sc0:sc0 + 1].to_broadcast([P, P]),
                            op=mybir.AluOpType.is_equal)
                        ind = blkp.tile([P, nsc_blk * 64], f16, tag="ind")
                        in0 = bass.AP(iota64.tensor, iota64[:].offset,
                                      [iota64[:].ap[0], [0, nsc_blk],
                                       [1, 64]])
                        in1 = bass.AP(dstl_sb.tensor,
                                      dstl_sb[:, sc0:sc0 + 1].offset,
                                      [dstl_sb[:].ap[0], [1, nsc_blk],
                                       [0, 64]])
                        nc.vector.tensor_tensor(out=ind[:], in0=in0, in1=in1,
                                                op=mybir.AluOpType.is_equal)
                        # asrc recompute from gathered rows
                        asrc = blkp.tile([P, nsc_blk * na], f32, tag="asrc")
                        for half in range(2):
                            gt = lt if half == 0 else ht
                            jl0 = bi * scW
                            tmp = blkp.tile([P, scW * P], f16,
                                            tag=f"tmp{half}")
                            nc.vector.tensor_tensor(
                                out=tmp[:],
                                in0=bass.AP(gt.tensor,
                                            gt[:].offset + jl0 * EL,
                                            [gt[:].ap[0], [EL, scW],
                                             [1, P]]),
                                in1=bass.AP(aW_l.tensor, aW_l.offset,
                                            [aW_l.ap[0], [0, scW], [1, P]]),
                                op=mybir.AluOpType.mult)
                            o_ap = bass.AP(
                                asrc.tensor,
                                asrc[:].offset + half * scW * na,
                                [asrc[:].ap[0], [1, scW * na]])
                            if layer < 2:
                                i_ap = bass.AP(
                                    tmp.tensor, tmp[:].offset,
                                    [tmp[:].ap[0], [P, scW], [CH, HEADS],
                                     [1, CH]])
                            else:
                                i_ap = bass.AP(
                                    tmp.tensor, tmp[:].offset,
                                    [tmp[:].ap[0], [P, scW], [1, P]])
                            nc.vector.tensor_reduce(
                                o_ap, i_ap, axis=mybir.AxisListType.X,
                                op=mybir.AluOpType.add)
                        # adst via indT matmuls
                        eadst = apsum.tile([P, nsc_blk * na], f32,
                                           space="PSUM", tag="eadst")
                        for s in range(nsc_blk):
                            j = (bi * nsc_blk + s) * P
                            nc.tensor.matmul(
                                out=eadst[:, s * na:(s + 1) * na],
                                lhsT=indT[:, j:j + P],
                                rhs=adst_all[layer][:,
                                                    blk * na:(blk + 1) * na],
                                start=True, stop=True,
                                skip_group_check=True)
                        # e = leaky_relu(asrc + adst); ee = exp(e)
                        et = blkp.tile([P, nsc_blk * na], f32, tag="et")
                        nc.vector.tensor_tensor(out=et[:], in0=asrc[:],
                                                in1=eadst[:],
                                                op=mybir.AluOpType.add)
                        elr = blkp.tile([P, nsc_blk * na], f32, tag="elr")
                        nc.vector.scalar_tensor_tensor(
                            out=elr[:], in0=et[:], scalar=NEG_SLOPE,
                            in1=et[:], op0=mybir.AluOpType.mult,
                            op1=mybir.AluOpType.max)
                        ee = blkp.tile([P, nsc_blk * na], f16, tag="ee")
                        nc.scalar.activation(ee[:], elr[:],
                                             mybir.ActivationFunctionType.Exp)
                        # macc = [h * ee | ee] per sub-chunk
                        macc = blkp.tile([P, nsc_blk * mw], f16, tag="macc")
                        for half in range(2):
                            gt = lt if half == 0 else ht
                            jl0 = bi * scW
                            nc.vector.tensor_tensor(
                                out=bass.AP(
                                    macc.tensor,
                                    macc[:].offset + half * scW * mw,
                                    [macc[:].ap[0], [mw, scW], [1, hw]]),
                                in0=bass.AP(gt.tensor,
                                            gt[:].offset + jl0 * EL,
                                            [gt[:].ap[0], [EL, scW],
                                             [1, hw]]),
                                in1=bass.AP(ee.tensor,
                                            ee[:].offset + half * scW * na,
                                            [ee[:].ap[0], [1, scW * na],
                                             [0, chw]]),
                                op=mybir.AluOpType.mult)
                            nc.vector.tensor_copy(
                                bass.AP(
                                    macc.tensor,
                                    macc[:].offset + half * scW * mw + hw,
                                    [macc[:].ap[0], [mw, scW], [1, na]]),
                                bass.AP(ee.tensor,
                                        ee[:].offset + half * scW * na,
                                        [ee[:].ap[0], [na, scW], [1, na]]))
                        # accumulate [msg | den] into PSUM
                        acc = apsum.tile([P, mw], f32, space="PSUM",
                                         tag="acc")
                        for s in range(nsc_blk):
                            first = s == 0
                            last = s == nsc_blk - 1
                            if first:
                                lhs = indf[:]
                                rows = acc[:, :]
                            else:
                                w = (s % scW) // S
                                lhs = ind[:, s * 64:(s + 1) * 64]
                                rows = acc[w * 64:(w + 1) * 64, :]
                            nc.tensor.matmul(
                                out=rows[:, 0:mw],
                                lhsT=lhs,
                                rhs=macc[:, s * mw:(s + 1) * mw],
                                start=first, stop=last,
                                skip_group_check=True)
                        # finalize block
                        den = blkp.tile([P, na], f32, tag="den")
                        nc.vector.tensor_scalar_add(
                            den[:], acc[:, hw:hw + na], 1e-16)
                        rec = blkp.tile([P, na], f32, tag="rec")
                        nc.vector.reciprocal(rec[:], den[:])
                        xo = blkp.tile([P, hw], f32, tag="xo")
                        rec_b = bass.AP(rec.tensor, rec[:].offset,
                                        [rec[:].ap[0], [1, na], [0, chw]])
                        nc.vector.tensor_tensor(out=xo[:], in0=acc[:, 0:hw],
                                                in1=rec_b,
                                                op=mybir.AluOpType.mult)
                        if tau_t[layer] is not None:
                            nc.vector.tensor_tensor(
                                out=xo[:], in0=xo[:], in1=tau_t[layer][:],
                                op=mybir.AluOpType.add)
                        if layer < 2:
                            ng = blkp.tile([P, hw], f32, tag="ng")
                            nc.vector.tensor_scalar_min(ng[:], xo[:], 0.0)
                            en = blkp.tile([P, hw], f32, tag="en")
                            nc.scalar.activation(
                                en[:], ng[:],
                                mybir.ActivationFunctionType.Exp)
                            ps = blkp.tile([P, hw], f32, tag="ps")
                            nc.vector.tensor_scalar_max(ps[:], xo[:], 0.0)
                            xe = blkp.tile([P, hw], f16, tag="xe")
                            nc.vector.scalar_tensor_tensor(
                                out=xe[:], in0=en[:], scalar=-1.0,
                                in1=ps[:], op0=mybir.AluOpType.add,
                                op1=mybir.AluOpType.add)
                            nc.sync.dma_start(
                                xbuf[layer][blk * P:(blk + 1) * P, :], xe[:])
                        else:
                            # int8 quantize with per-row scale
                            rmax = blkp.tile([P, 1], f32, tag="rmax")
                            nc.vector.tensor_reduce(
                                rmax[:], xo[:], axis=mybir.AxisListType.X,
                                op=mybir.AluOpType.max,
                                apply_absolute_value=True)
                            rmx = blkp.tile([P, 1], f32, tag="rmx")
                            nc.vector.tensor_scalar_max(rmx[:], rmax[:],
                                                        1e-6)
                            rs = blkp.tile([P, 1], f32, tag="rs")
                            nc.vector.reciprocal(rs[:], rmx[:])
                            yqf = blkp.tile([P, OUT], f32, tag="yqf")
                            nc.vector.scalar_tensor_tensor(
                                out=yqf[:], in0=xo[:], scalar=127.0,
                                in1=bass.AP(rs.tensor, rs[:].offset,
                                            [rs[:].ap[0], [0, OUT]]),
                                op0=mybir.AluOpType.mult,
                                op1=mybir.AluOpType.mult)
                            yo = blkp.tile([P, OUT + 2], i8, tag="yo")
                            nc.vector.tensor_copy(yo[:, 0:OUT], yqf[:])
                            nc.vector.tensor_scalar_mul(
                                yo[:, OUT:OUT + 2].bitcast(f16), rmx[:],
                                1.0 / 127.0)
                            nc.sync.dma_start(
                                yq[blk * P:(blk + 1) * P, :], yo[:])
                    b0 += nb
                    offL += nL * 8
                    offT += nA * P

            nlayers = int(os.environ.get("K_LAYERS", "3"))
            do_edge = os.environ.get("K_EDGE", "1") == "1"
            nrep = int(os.environ.get("K_REPEAT", "1"))
            for _rep in range(nrep):
                for layer in range(nlayers):
                    node_phase(layer)
                    if do_edge:
                        edge_phase(layer)
            if nlayers < 3 or not do_edge:
                zt = blkp.tile([P, OUT + 2], i8, tag="ytouch")
                nc.gpsimd.memset(zt[:], 0.0)
                for blk in range(BPC):
                    nc.sync.dma_start(yq[blk * P:(blk + 1) * P, :], zt[:])
    nc.compile()
    return nc


# ---------------------------------------------------------------------------
# numpy mirror of the device pipeline (for plan/stream validation)
# ---------------------------------------------------------------------------

def numpy_pipeline(pl, x, Wcs, aWs, taus):
    cfg = pl["cfg"]
    perm = pl["perm"]
    xp = np.zeros((cfg.NPAD, cfg.IN_CH), np.float32)
    xp[perm] = x
    S = cfg.S
    nsc_blk = 4 * S
    for layer in range(3):
        hw = cfg.HID if layer < 2 else cfg.OUT_CH
        na = cfg.HEADS if layer < 2 else 1
        chw = cfg.CH if layer < 2 else cfg.OUT_CH
        h = (xp @ Wcs[layer]).astype(np.float32)   # [NPAD, hw+na]
        table = np.zeros((cfg.NPAD, EL), np.float16)
        table[:, 0:hw] = h[:, 0:hw].astype(np.float16)
        adst16 = h[:, hw:hw + na].astype(np.float16)
        aW = aWs[layer]
        xnew = np.zeros((cfg.NPAD, hw), np.float32)
        for c in range(cfg.NCORES):
            gidx, dstl, dtv = pl["gidx"][c], pl["dstl"][c], pl["dtv"][c]
            for blk in range(cfg.BPC):
                accw = np.zeros((P, hw + na), np.float32)
                adst_blk = adst16[c * cfg.NPC + blk * P:
                                  c * cfg.NPC + (blk + 1) * P]
                for s in range(nsc_blk):
                    sc = blk * nsc_blk + s
                    half = 0 if s < 2 * S else 1
                    w = (s % (2 * S)) // S
                    gi = gidx[sc] + (cfg.HALFN if half else 0)
                    rows = table[gi]                         # [P, EL] f16
                    # grouped reduce: heads of width hw//na (aW zero-padded)
                    asrc = (rows.astype(np.float32) * aW[None, :])[
                        :, 0:hw].reshape(P, na, hw // na).sum(-1)
                    dv = dtv[sc].astype(np.int64)
                    ad = np.where(dv[:, None] >= 0,
                                  adst_blk[np.maximum(dv, 0)].astype(
                                      np.float32), 0.0)
                    e = asrc + ad
                    e = np.maximum(e, NEG_SLOPE * e)
                    ee = np.exp(e).astype(np.float16)
                    msg = (rows[:, 0:hw] *
                           np.repeat(ee, chw, axis=1)).astype(np.float16)
                    sl = dstl[sc]
                    mrow = np.concatenate(
                        [msg.astype(np.float32), ee.astype(np.float32)], 1)
                    if s == 0:
                        indm = (np.arange(P)[None, :] == sl[:, None])
                        accw += indm.astype(np.float32).T @ mrow
                    else:
                        indm = (np.arange(64)[None, :] == sl[:, None])
                        accw[w * 64:(w + 1) * 64] += \
                            indm.astype(np.float32).T @ mrow
                rec = 1.0 / (accw[:, hw:hw + na] + 1e-16)
                xo = accw[:, 0:hw] * np.repeat(rec, chw, axis=1)
                if taus[layer] is not None:
                    xo = xo + taus[layer][None, :]
                if layer < 2:
                    xo = np.where(xo > 0, xo,
                                  np.exp(np.minimum(xo, 0)) - 1)
                xnew[c * cfg.NPC + blk * P:c * cfg.NPC + (blk + 1) * P] = \
                    xo.astype(np.float16).astype(np.float32)
        xp = xnew
    return xp[perm]


# ---------------------------------------------------------------------------
# public entry
# ---------------------------------------------------------------------------

_CACHE = {}


def _get_program(cfg_key, cfg, with_tau):
    key = (cfg_key, with_tau, cfg.S)
    if key not in _CACHE:
        _CACHE[key] = build_program(cfg, with_tau)
    return _CACHE[key]


def prep_inputs(cfg, inputs):
    """Host planning + per-core input maps. Returns (nc, in_maps, perm)."""
    x = np.asarray(inputs["x"], np.float32)
    edge_index = np.asarray(inputs["edge_index"])
    pl = plan(cfg, edge_index)
    streams = make_streams(pl)
    Wc0, aW0, tau0 = fold_weights(
        inputs["W0"], inputs["a_src0"], inputs["a_dst0"], inputs["bn_g0"],
        inputs["bn_b0"], inputs["bn_m0"], inputs["bn_v0"], inputs["b0"])
    Wc1, aW1, tau1 = fold_weights(
        inputs["W1"], inputs["a_src1"], inputs["a_dst1"], inputs["bn_g1"],
        inputs["bn_b1"], inputs["bn_m1"], inputs["bn_v1"], inputs["b1"])
    Wc2, aW2, tau2 = fold_weights(
        inputs["W2"], inputs["a_src2"], inputs["a_dst2"], bias=inputs["b2"])
    taus = [tau0, tau1, tau2]
    with_tau = tuple(bool(np.any(t != 0)) for t in taus)
    nc = _get_program("gat", cfg, with_tau)

    perm = pl["perm"]
    OFF = blob_layout(cfg)
    aW_row = np.concatenate([aW0, aW1, aW2]).astype(np.float16)
    xs = np.maximum(np.abs(x).max(axis=0), 1e-9) / 127.0  # per input channel
    wc16 = [(Wc0 * xs[:, None]).astype(np.float16).ravel(),
            Wc1.astype(np.float16).ravel(),
            Wc2.astype(np.float16).ravel()]
    in_maps = []
    for c in range(cfg.NCORES):
        xc = np.zeros((cfg.NPC, cfg.IN_CH), np.float32)
        mask = (perm >= c * cfg.NPC) & (perm < (c + 1) * cfg.NPC)
        origs = np.nonzero(mask)[0]
        xc[perm[origs] - c * cfg.NPC] = x[origs]
        xcT = np.ascontiguousarray(xc.T)
        s = streams[c]
        blob = np.concatenate([
            pack_x8(xcT, xs).ravel().view(np.float16),
            wc16[0], wc16[1], wc16[2], aW_row,
            s["idxL"].ravel().view(np.float16),
            s["idxH"].ravel().view(np.float16),
            s["dstl"].ravel().view(np.float16),
            s["dstlT"].ravel().view(np.float16),
        ])[None, :]
        assert blob.shape[1] == OFF["TOT"], (blob.shape, OFF["TOT"])
        m = dict(blob=blob)
        for li, t in enumerate(taus):
            if with_tau[li]:
                m[f"tau{li}"] = t[None, :].astype(np.float32)
        in_maps.append(m)
    return nc, in_maps, perm, (pl, [Wc0, Wc1, Wc2], [aW0, aW1, aW2], taus,
                               with_tau)


def unquant_y(results, ncores):
    """[core]{yq} (int8 rows [q(40)|f16 scale]) -> full [NPAD, OUT] f32."""
    outs = []
    for c in range(ncores):
        raw = np.ascontiguousarray(results[c]["yq"])
        nout = raw.shape[1] - 2
        q = raw[:, 0:nout].astype(np.float32)
        s = raw[:, nout:nout + 2].copy().view(np.float16).astype(np.float32)
        outs.append(q * s)
    return np.concatenate(outs, axis=0)


def run(cfg, inputs, trace=False):
    nc, in_maps, perm, _ = prep_inputs(cfg, inputs)
    res = run_bass_kernel_spmd(nc, in_maps, list(range(cfg.NCORES)),
                               trace=trace)
    out = unquant_y(res.results, cfg.NCORES)[perm]
    return out, res


def kernel(**inputs):
    cfg = Cfg(n=50000, e=800000)
    out, _ = run(cfg, inputs)
    return out.astype(np.float32)


# revision 54
# speedup vs baseline: 1.1828x; 1.0123x over previous
"""GAT (3-layer) Trainium2 Bass kernel, 8-core SPMD.

Schedule:
 - Nodes are relabeled and packed by the host into a uniform schedule:
   8 cores x B blocks x 2 windows x 64 slots. Each (window, src-half) gets a
   fixed number S of 128-edge sub-chunks; all per-core variation lives in
   data (gather indices / dst-slot metadata), so one SPMD program serves all
   cores. Per layer: node phase (h = x @ Wc, AllGather of node tables),
   then edge phase (bulk dma_gather of h[src] rows, segment softmax +
   weighted sums via indicator matmuls accumulating in PSUM).

Data-plane design (end-to-end I/O is the dominant cost):
 - Node tables are [NPAD, 128] f16 (256B rows == minimum dma_gather
   granularity): row = h_tilde = (x @ W.T) * bn_scale (BN folded on host).
 - asrc is recomputed per edge on-chip: grouped reduce over the gathered
   h_tilde row against aW = a_src/bn_scale. No asrc in the table rows.
 - adst never hits DRAM: per block the 128 local-node adst values (kept in
   SBUF from the node phase) are spread to edge lanes with an
   indicator-transpose matmul, indT = is_equal(iota_p, dstlT).
 - Per sub-chunk ONE accumulation matmul: rhs = [h*ee | ee] so messages and
   softmax denominators accumulate together in PSUM.
 - All inputs ship as ONE packed blob per core (~1.4 MB): x quantized to
   int8 with per-input-channel scales folded into Wc0 (one DMA + one DVE
   add to unpack), de-replicated [16, W] int16 index streams (tiled to
   [128, W] by DMA), int8 dstl/dstlT, f16 weights.
 - Output is ONE int8 array per core: rows [q(40) | per-row f16 scale
   bitcast into the last two bytes], dequantized on host.
"""
import os
import numpy as np

os.environ.setdefault("JAX_COMPILATION_CACHE_DIR", "/tmp/jaxcache")

import concourse.bass as bass
import concourse.bacc as bacc
import concourse.tile as tile
import concourse.mybir as mybir
from concourse.bass_utils import run_bass_kernel_spmd
from concourse.masks import make_identity

P = 128
f32 = mybir.dt.float32
f16 = mybir.dt.float16
i16 = mybir.dt.int16
i8 = mybir.dt.int8
u8 = mybir.dt.uint8
u16 = mybir.dt.uint16

NEG_SLOPE = 0.2
BN_EPS = 1e-5
EL = 128            # f16 elems per table row (256B = min gather granularity)


class Cfg:
    def __init__(self, n, e, ncores=8, bpc=None, s=None, batch_blocks=4,
                 heads=8, ch=16, out_ch=40, in_ch=128):
        self.N = n
        self.E = e
        self.NCORES = ncores
        self.IN_CH = in_ch
        self.HID = heads * ch
        self.HEADS = heads
        self.CH = ch
        self.OUT_CH = out_ch
        assert n % ncores == 0
        self.npc_real = n // ncores
        self.BPC = bpc if bpc else (self.npc_real + P - 1) // P
        self.NPC = self.BPC * P              # node slots per core
        self.NPAD = self.NPC * ncores        # total node slots
        self.HALFN = self.NPAD // 2          # table half size
        assert ncores % 2 == 0
        assert self.HALFN < 32768, "half table must be int16 indexable"
        self.S = s
        bb = []
        nb = self.BPC
        while nb > 0:
            take = min(batch_blocks, nb)
            bb.append(take)
            nb -= take
        self.batches = bb


def _pack_core(deg_l, deg_h, nodes, bpc, cap):
    """Greedy 2D bin packing: nodes (orig ids) -> window. bins = bpc*2
    windows with 64 slots, capacity cap on both L and H edge sums."""
    nwin = bpc * 2
    rem_l = np.full(nwin, cap, np.int64)
    rem_h = np.full(nwin, cap, np.int64)
    slots = np.full(nwin, 64, np.int64)
    assign = np.empty(len(nodes), np.int64)
    order = np.argsort(-(deg_l[nodes] + deg_h[nodes]), kind="stable")
    for i in order:
        n = nodes[i]
        dl, dh = deg_l[n], deg_h[n]
        ok = (slots > 0) & (rem_l >= dl) & (rem_h >= dh)
        if not ok.any():
            return None
        score = np.where(ok, np.minimum(rem_l - dl, rem_h - dh), -1)
        w = int(np.argmax(score))
        assign[i] = w
        rem_l[w] -= dl
        rem_h[w] -= dh
        slots[w] -= 1
    return assign


def plan(cfg, edge_index):
    """Host planning. Returns dict with relabeling and per-core streams."""
    n, ncores = cfg.N, cfg.NCORES
    src = np.asarray(edge_index[0], np.int64)
    dst = np.asarray(edge_index[1], np.int64)
    loops = np.arange(n, dtype=np.int64)
    src_all = np.concatenate([src, loops])
    dst_all = np.concatenate([dst, loops])

    core_of = src_all // cfg.npc_real
    is_high = core_of >= (ncores // 2)
    deg_l = np.bincount(dst_all[~is_high], minlength=n)
    deg_h = np.bincount(dst_all[is_high], minlength=n)

    if cfg.S is None:
        mean = (len(src_all) / (ncores * cfg.BPC * 2 * 2))
        cfg.S = max(1, int(np.ceil(mean * 1.18 / P)))
    while True:
        cap = cfg.S * P
        assigns = []
        ok = True
        for c in range(ncores):
            nodes = np.arange(c * cfg.npc_real, (c + 1) * cfg.npc_real)
            a = _pack_core(deg_l, deg_h, nodes, cfg.BPC, cap)
            if a is None:
                ok = False
                break
            assigns.append(a)
        if ok:
            break
        cfg.S += 1

    S = cfg.S
    perm = np.empty(n, np.int64)  # orig -> new
    for c in range(ncores):
        nodes = np.arange(c * cfg.npc_real, (c + 1) * cfg.npc_real)
        a = assigns[c]
        used = np.zeros(cfg.BPC * 2, np.int64)
        for i, nd in enumerate(nodes):
            w = a[i]
            s = used[w]
            used[w] += 1
            perm[nd] = c * cfg.NPC + (w // 2) * P + (w % 2) * 64 + s
    src_new = perm[src_all]
    dst_new = perm[dst_all]

    e_core = dst_new // cfg.NPC
    e_local = dst_new % cfg.NPC
    e_blk = e_local // P
    e_win = (e_local % P) // 64
    e_slot = e_local % 64
    e_bslot = e_local % P            # block-wide slot 0..127
    e_half = (src_new >= cfg.HALFN).astype(np.int64)

    nsc_blk = 4 * S
    nsc_core = cfg.BPC * nsc_blk
    key = ((e_core * cfg.BPC + e_blk) * 2 + e_win) * 2 + e_half
    order = np.argsort(key, kind="stable")
    sorted_e = order
    key_sorted = key[order]
    nbuck = ncores * cfg.BPC * 2 * 2
    counts = np.bincount(key_sorted, minlength=nbuck)
    starts = np.concatenate([[0], np.cumsum(counts)])
    assert counts.max() <= S * P, f"bucket overflow {counts.max()} > {S*P}"

    gidx = np.zeros((ncores, nsc_core, P), np.int64)       # table row (half)
    dstl = np.full((ncores, nsc_core, P), -1.0, np.float32)  # window slot
    dtv = np.full((ncores, nsc_core, P), -1.0, np.float32)   # block slot
    for c in range(ncores):
        for b in range(cfg.BPC):
            for h in range(2):
                for w in range(2):
                    bucket = ((c * cfg.BPC + b) * 2 + w) * 2 + h
                    lo, hi = starts[bucket], starts[bucket + 1]
                    ee = sorted_e[lo:hi]
                    sc0 = b * nsc_blk + h * 2 * S + w * S
                    k = np.arange(hi - lo)
                    scs = sc0 + k // P
                    lanes = k % P
                    gi = src_new[ee] - (cfg.HALFN if h else 0)
                    gidx[c, scs, lanes] = gi
                    dstl[c, scs, lanes] = e_slot[ee]
                    dtv[c, scs, lanes] = e_bslot[ee]
    return dict(cfg=cfg, perm=perm, gidx=gidx, dstl=dstl, dtv=dtv,
                src_all=src_all, dst_all=dst_all)


def _wrap_idx(vals):
    """vals [NI] int -> wrapped [16, NI/16] int16."""
    ni = len(vals)
    assert ni % 128 == 0
    w = np.zeros((16, ni // 16), np.int16)
    w[np.arange(ni) % 16, np.arange(ni) // 16] = vals.astype(np.int16)
    return w


def make_streams(pl):
    """Per-core input arrays for the device program."""
    cfg = pl["cfg"]
    S, BPC = cfg.S, cfg.BPC
    nsc_blk = 4 * S
    out = []
    for c in range(cfg.NCORES):
        gidx, dstl, dtv = pl["gidx"][c], pl["dstl"][c], pl["dtv"][c]
        idxL_b, idxH_b = [], []
        b0 = 0
        for nb in cfg.batches:
            scs = np.arange(b0 * nsc_blk, (b0 + nb) * nsc_blk)
            b0 += nb
            blk = scs.reshape(nb, 4 * S)
            l_scs = blk[:, :2 * S].ravel()
            h_scs = blk[:, 2 * S:].ravel()
            idxL_b.append(_wrap_idx(gidx[l_scs].ravel()))
            idxH_b.append(_wrap_idx(gidx[h_scs].ravel()))
        out.append(dict(
            idxL=np.concatenate(idxL_b, axis=1),
            idxH=np.concatenate(idxH_b, axis=1),
            dstl=np.ascontiguousarray(dstl.T).astype(np.int8),  # [128, nsc]
            dstlT=dtv.ravel()[None, :].astype(np.int8),  # [1, nsc*128]
        ))
    return out


def blob_layout(cfg):
    """f16-element offsets of each section in the per-core input blob."""
    nsc_core = cfg.BPC * 4 * cfg.S
    nsc_l = nsc_core // 2
    W01 = cfg.HID + cfg.HEADS
    W2C = cfg.OUT_CH + 1
    off = {}
    o = 0
    for name, sz in (("xP", P * cfg.NPC // 2), ("Wc0", P * W01),
                     ("Wc1", P * W01),
                     ("Wc2", P * W2C), ("aW", 3 * P),
                     ("idxL", 16 * nsc_l * 8), ("idxH", 16 * nsc_l * 8),
                     ("dstl", P * nsc_core // 2),
                     ("dstlT", nsc_core * P // 2)):
        off[name] = o
        o += sz
    off["TOT"] = o
    return off


def pack_x8(xcT, s_vec):
    """[P, NPC] f32 channel-major -> per-channel int8, offset-binary u8."""
    q = np.clip(np.round(xcT / s_vec[:, None]), -128, 127) + 128
    return q.astype(np.uint8)


def fold_weights(W, a_src, a_dst, bn_g=None, bn_b=None, bn_m=None, bn_v=None,
                 bias=None):
    """Build Wc [in, hw+na] (h_tilde | adst cols), aW [128] (a_src/bn_scale,
    zero-padded), and shift tau [hw]."""
    W = np.asarray(W, np.float64)
    heads, ch = np.asarray(a_src).shape
    out_ch = W.shape[0]
    if bn_g is not None:
        s = np.asarray(bn_g, np.float64) / np.sqrt(
            np.asarray(bn_v, np.float64) + BN_EPS)
        t = np.asarray(bn_b, np.float64) - np.asarray(bn_m, np.float64) * s
    else:
        s = np.ones(out_ch)
        t = np.zeros(out_ch)
    tau = (np.asarray(bias, np.float64) * s + t) if bias is not None else t
    Wt = W.T * s[None, :]                     # [in, out] scaled
    adst_col = np.zeros((W.shape[1], heads))
    for h in range(heads):
        adst_col[:, h] = W[h * ch:(h + 1) * ch, :].T @ \
            np.asarray(a_dst, np.float64)[h]
    Wc = np.concatenate([Wt, adst_col], axis=1).astype(np.float32)
    aW = np.zeros(P, np.float32)
    aW[:out_ch] = (np.asarray(a_src, np.float64).ravel() / s).astype(
        np.float32)
    return Wc, aW, tau.astype(np.float32)


# ---------------------------------------------------------------------------
# device program
# ---------------------------------------------------------------------------

def build_program(cfg, with_tau=(False, False, False)):
    S, BPC, NCORES = cfg.S, cfg.BPC, cfg.NCORES
    HEADS, CH, OUT = cfg.HEADS, cfg.CH, cfg.OUT_CH
    HID = cfg.HID
    NPC, NPAD, HALFN = cfg.NPC, cfg.NPAD, cfg.HALFN
    nsc_blk = 4 * S
    nsc_core = BPC * nsc_blk
    W01 = HID + HEADS       # 136
    W2C = OUT + 1           # 41

    nc = bacc.Bacc("TRN2", target_bir_lowering=False, debug=False,
                   num_devices=NCORES)

    # ---- inputs (one packed blob; see blob_layout) ----
    nsc_l = nsc_core // 2
    OFF = blob_layout(cfg)
    blob = nc.dram_tensor("blob", [1, OFF["TOT"]], f16, kind="ExternalInput")
    taus = []
    for li in range(3):
        if with_tau[li]:
            w = HID if li < 2 else OUT
            taus.append(nc.dram_tensor(f"tau{li}", [1, w], f32,
                                       kind="ExternalInput"))
        else:
            taus.append(None)

    # int8 rows [q(40) | f16 scale bitcast into cols 40:42]
    yq = nc.dram_tensor("yq", [NPC, OUT + 2], i8, kind="ExternalOutput")

    # ---- internal DRAM ----
    tbl_slice = [nc.dram_tensor(f"tbs{i}", [NPC, EL], f16) for i in range(3)]
    tbl_full = [nc.dram_tensor(f"tbf{i}", [NPAD, EL], f16) for i in range(3)]
    xbuf = [nc.dram_tensor(f"xb{i}", [NPC, HID], f16) for i in range(2)]

    with tile.TileContext(nc) as tc:
        import contextlib
        ctx = contextlib.ExitStack()
        with ctx:
            const = ctx.enter_context(tc.tile_pool(name="const", bufs=1))
            nodep = ctx.enter_context(tc.tile_pool(name="nodep", bufs=2))
            npsum = ctx.enter_context(
                tc.tile_pool(name="npsum", bufs=2, space="PSUM"))
            gath = ctx.enter_context(tc.tile_pool(name="gath", bufs=2))
            blkp = ctx.enter_context(tc.tile_pool(name="blkp", bufs=2))
            apsum = ctx.enter_context(
                tc.tile_pool(name="apsum", bufs=2, space="PSUM"))

            # constants
            iota64 = const.tile([P, 64], f16)
            nc.gpsimd.iota(iota64[:], pattern=[[1, 64]], base=0,
                           channel_multiplier=0,
                           allow_small_or_imprecise_dtypes=True)
            iota128 = const.tile([P, P], f16)
            nc.gpsimd.iota(iota128[:], pattern=[[1, 128]], base=0,
                           channel_multiplier=0,
                           allow_small_or_imprecise_dtypes=True)
            iotaPP = const.tile([P, 1], i8)
            nc.gpsimd.iota(iotaPP[:], pattern=[[0, 1]], base=0,
                           channel_multiplier=1,
                           allow_small_or_imprecise_dtypes=True)
            ident = const.tile([P, P], f16)
            make_identity(nc, ident[:])
            wc_t = []
            for nm, wdt, dt_ in (("Wc0", W01, f16), ("Wc1", W01, f16),
                                 ("Wc2", W2C, f16)):
                w_sb = const.tile([P, wdt], dt_, tag=f"wc{nm}")
                nc.sync.dma_start(
                    w_sb[:], bass.AP(blob, OFF[nm], [[wdt, P], [1, wdt]]))
                wc_t.append(w_sb)
            aW_sb = const.tile([P, 3 * P], f16)
            nc.sync.dma_start(
                aW_sb[:], bass.AP(blob, OFF["aW"], [[0, P], [1, 3 * P]]))
            dstl8 = const.tile([P, nsc_core], i8)
            nc.sync.dma_start(
                dstl8[:], bass.AP(blob, OFF["dstl"],
                                  [[nsc_core // 2, P],
                                   [1, nsc_core // 2]]).bitcast(i8))
            dstl_sb = const.tile([P, nsc_core], f16)
            nc.vector.tensor_copy(dstl_sb[:], dstl8[:])
            # unpack 12-bit packed x -> xt_all [P, NPC] f16 (= round(x/s),
            # offset removed; the x scale s is folded into Wc0 on host)
            # unpack offset-binary u8 x -> xt_all f16 (per-channel scales are
            # folded into Wc0 on host, so xt holds round(x/s_c) exactly)
            xt_all = const.tile([P, NPC], f16, tag="xt_all")
            with tc.tile_pool(name="unpk", bufs=1) as unpk:
                xb = unpk.tile([P, NPC], u8, tag="xb")
                nc.sync.dma_start(
                    xb[:], bass.AP(blob, OFF["xP"],
                                   [[NPC // 2, P],
                                    [1, NPC // 2]]).bitcast(u8))
                nc.vector.tensor_scalar_add(xt_all[:], xb[:], -128.0)
            tau_t = []
            for li in range(3):
                if taus[li] is not None:
                    w = HID if li < 2 else OUT
                    tt = const.tile([P, w], f32, tag=f"tau{li}")
                    nc.sync.dma_start(
                        tt[:], bass.AP(taus[li], 0, [[0, P], [1, w]]))
                    tau_t.append(tt)
                else:
                    tau_t.append(None)
            # per-layer adst of local nodes (written in node phase, read in
            # edge phase; never leaves SBUF)
            adst0 = const.tile([P, BPC * HEADS], f16, tag="adst0")
            adst1 = const.tile([P, BPC * HEADS], f16, tag="adst1")
            adst2 = const.tile([P, BPC], f16, tag="adst2")
            adst_all = [adst0, adst1, adst2]

            def node_phase(layer):
                wdt = W01 if layer < 2 else W2C
                na = HEADS if layer < 2 else 1
                hw = HID if layer < 2 else OUT
                for t in range(BPC):
                    if layer == 0:
                        xt_ap = xt_all[:, t * P:(t + 1) * P]
                    else:
                        xin = nodep.tile([P, P], f16, tag="xin")
                        nc.sync.dma_start(
                            xin[:], xbuf[layer - 1][t * P:(t + 1) * P, :])
                        xtp = npsum.tile([P, P], f16, space="PSUM", tag="xtp")
                        nc.tensor.transpose(out=xtp[:], in_=xin[:],
                                            identity=ident[:])
                        xt = nodep.tile([P, P], f16, tag="xt16")
                        nc.vector.tensor_copy(xt[:], xtp[:])
                        xt_ap = xt[:]
                    hps = npsum.tile([P, wdt], f32, space="PSUM", tag="hps")
                    nc.tensor.matmul(out=hps[:], lhsT=xt_ap,
                                     rhs=wc_t[layer][:],
                                     start=True, stop=True)
                    hx16 = nodep.tile([P, EL], f16, tag="hx16")
                    nc.vector.tensor_copy(hx16[:, 0:hw], hps[:, 0:hw])
                    nc.sync.dma_start(
                        tbl_slice[layer][t * P:(t + 1) * P, :], hx16[:])
                    nc.vector.tensor_copy(
                        adst_all[layer][:, t * na:(t + 1) * na],
                        hps[:, hw:hw + na])
                if os.environ.get("K_NOCOLL", "0") != "1":
                    nc.gpsimd.collective_compute(
                        "AllGather", mybir.AluOpType.bypass,
                        replica_groups=[list(range(NCORES))],
                        ins=[tbl_slice[layer][:, :]],
                        outs=[tbl_full[layer][:, :]])

            g_chunk = int(os.environ.get("K_GCHUNK", "16"))
            g_sp = os.environ.get("K_SP", "0") == "1"

            def do_gather(out_tile, table_ap, idx_tile, n_sc, el):
                for c0 in range(0, n_sc, g_chunk):
                    cn = min(g_chunk, n_sc - c0)
                    o_ap = bass.AP(out_tile.tensor,
                                   out_tile[:].offset + c0 * el,
                                   [out_tile[:].ap[0], [el, cn], [1, el]])
                    i_ap = bass.AP(idx_tile.tensor,
                                   idx_tile[:].offset + c0 * 8,
                                   [idx_tile[:].ap[0], [1, cn * 8]])
                    nc.gpsimd.dma_gather(
                        out_ap=o_ap, in_ap=table_ap, idxs_ap=i_ap,
                        num_idxs=cn * P, num_idxs_reg=cn * P,
                        elem_size=el, single_packet=g_sp)

            def edge_phase(layer):
                na = HEADS if layer < 2 else 1
                hw = HID if layer < 2 else OUT
                chw = CH if layer < 2 else OUT
                mw = hw + na                 # macc row width per sub-chunk
                full = tbl_full[layer]
                aW_l = aW_sb[:, layer * P:(layer + 1) * P]
                scW = 2 * S
                b0 = 0
                offL = 0
                offT = 0
                for nb in cfg.batches:
                    nL = nb * scW
                    nA = nb * nsc_blk
                    iL = gath.tile([P, nL * 8], i16, tag="iL")
                    nc.sync.dma_start(
                        iL[:], bass.AP(blob, OFF["idxL"] + offL,
                                       [[0, 8], [nsc_l * 8, 16],
                                        [1, nL * 8]]).bitcast(i16))
                    iH = gath.tile([P, nL * 8], i16, tag="iH")
                    nc.sync.dma_start(
                        iH[:], bass.AP(blob, OFF["idxH"] + offL,
                                       [[0, 8], [nsc_l * 8, 16],
                                        [1, nL * 8]]).bitcast(i16))
                    lt = gath.tile([P, nL * EL], f16, tag="lt")
                    do_gather(lt, full[0:HALFN, :], iL, nL, EL)
                    ht = gath.tile([P, nL * EL], f16, tag="ht")
                    do_gather(ht, full[HALFN:NPAD, :], iH, nL, EL)
                    # block-slot values replicated to all partitions + indT
                    dtr = gath.tile([P, nA * P], i8, tag="dtr")
                    nc.sync.dma_start(
                        dtr[:], bass.AP(blob, OFF["dstlT"] + offT // 2,
                                        [[0, P],
                                         [1, nA * P // 2]]).bitcast(i8))
                    indT = gath.tile([P, nA * P], f16, tag="indT")
                    nc.vector.tensor_tensor(
                        out=indT[:],
                        in0=iotaPP[:, 0:1].to_broadcast([P, nA * P]),
                        in1=dtr[:], op=mybir.AluOpType.is_equal)

                    for bi in range(nb):
                        blk = b0 + bi
                        sc0 = blk * nsc_blk
                        # indicators
                        indf = blkp.tile([P, P], f16, tag="indf")
                        nc.vector.tensor_tensor(
                            out=indf[:], in0=iota128[:],
                            in1=dstl_sb[:, sc0:sc0 + 1].to_broadcast([P, P]),
                            op=mybir.AluOpType.is_equal)
                        ind = blkp.tile([P, nsc_blk * 64], f16, tag="ind")
                        in0 = bass.AP(iota64.tensor, iota64[:].offset,
                                      [iota64[:].ap[0], [0, nsc_blk],
                                       [1, 64]])
                        in1 = bass.AP(dstl_sb.tensor,
                                      dstl_sb[:, sc0:sc0 + 1].offset,
                                      [dstl_sb[:].ap[0], [1, nsc_blk],
                                       [0, 64]])
                        nc.vector.tensor_tensor(out=ind[:], in0=in0, in1=in1,
                                                op=mybir.AluOpType.is_equal)
                        # asrc recompute from gathered rows
                        asrc = blkp.tile([P, nsc_blk * na], f32, tag="asrc")
                        for half in range(2):
                            gt = lt if half == 0 else ht
                            jl0 = bi * scW
                            tmp = blkp.tile([P, scW * P], f16,
                                            tag=f"tmp{half}")
                            nc.vector.tensor_tensor(
                                out=tmp[:],
                                in0=bass.AP(gt.tensor,
                                            gt[:].offset + jl0 * EL,
                                            [gt[:].ap[0], [EL, scW],
                                             [1, P]]),
                                in1=bass.AP(aW_l.tensor, aW_l.offset,
                                            [aW_l.ap[0], [0, scW], [1, P]]),
                                op=mybir.AluOpType.mult)
                            o_ap = bass.AP(
                                asrc.tensor,
                                asrc[:].offset + half * scW * na,
                                [asrc[:].ap[0], [1, scW * na]])
                            if layer < 2:
                                i_ap = bass.AP(
                                    tmp.tensor, tmp[:].offset,
                                    [tmp[:].ap[0], [P, scW], [CH, HEADS],
                                     [1, CH]])
                            else:
                                i_ap = bass.AP(
                                    tmp.tensor, tmp[:].offset,
                                    [tmp[:].ap[0], [P, scW], [1, P]])
                            nc.vector.tensor_reduce(
                                o_ap, i_ap, axis=mybir.AxisListType.X,
                                op=mybir.AluOpType.add)
                        # adst via indT matmuls
                        eadst = apsum.tile([P, nsc_blk * na], f32,
                                           space="PSUM", tag="eadst")
                        for s in range(nsc_blk):
                            j = (bi * nsc_blk + s) * P
                            nc.tensor.matmul(
                                out=eadst[:, s * na:(s + 1) * na],
                                lhsT=indT[:, j:j + P],
                                rhs=adst_all[layer][:,
                                                    blk * na:(blk + 1) * na],
                                start=True, stop=True,
                                skip_group_check=True)
                        # e = leaky_relu(asrc + adst); ee = exp(e)
                        et = blkp.tile([P, nsc_blk * na], f32, tag="et")
                        nc.vector.tensor_tensor(out=et[:], in0=asrc[:],
                                                in1=eadst[:],
                                                op=mybir.AluOpType.add)
                        elr = blkp.tile([P, nsc_blk * na], f32, tag="elr")
                        nc.vector.scalar_tensor_tensor(
                            out=elr[:], in0=et[:], scalar=NEG_SLOPE,
                            in1=et[:], op0=mybir.AluOpType.mult,
                            op1=mybir.AluOpType.max)
                        # macc = [h * ee | ee] per sub-chunk; exp writes its
                        # result directly into the strided ee columns, the
                        # msg mult reads it back from there (disjoint cols)
                        macc = blkp.tile([P, nsc_blk * mw], f16, tag="macc")
                        nc.scalar.activation(
                            bass.AP(macc.tensor, macc[:].offset + hw,
                                    [macc[:].ap[0], [mw, nsc_blk], [1, na]]),
                            elr[:], mybir.ActivationFunctionType.Exp)
                        for half in range(2):
                            gt = lt if half == 0 else ht
                            jl0 = bi * scW
                            nc.vector.tensor_tensor(
                                out=bass.AP(
                                    macc.tensor,
                                    macc[:].offset + half * scW * mw,
                                    [macc[:].ap[0], [mw, scW], [1, hw]]),
                                in0=bass.AP(gt.tensor,
                                            gt[:].offset + jl0 * EL,
                                            [gt[:].ap[0], [EL, scW],
                                             [1, hw]]),
                                in1=bass.AP(
                                    macc.tensor,
                                    macc[:].offset + half * scW * mw + hw,
                                    [macc[:].ap[0], [mw, scW], [1, na],
                                     [0, chw]]),
                                op=mybir.AluOpType.mult)
                        # accumulate [msg | den] into PSUM
                        acc = apsum.tile([P, mw], f32, space="PSUM",
                                         tag="acc")
                        for s in range(nsc_blk):
                            first = s == 0
                            last = s == nsc_blk - 1
                            if first:
                                lhs = indf[:]
                                rows = acc[:, :]
                            else:
                                w = (s % scW) // S
                                lhs = ind[:, s * 64:(s + 1) * 64]
                                rows = acc[w * 64:(w + 1) * 64, :]
                            nc.tensor.matmul(
                                out=rows[:, 0:mw],
                                lhsT=lhs,
                                rhs=macc[:, s * mw:(s + 1) * mw],
                                start=first, stop=last,
                                skip_group_check=True)
                        # finalize block
                        den = blkp.tile([P, na], f32, tag="den")
                        nc.vector.tensor_scalar_add(
                            den[:], acc[:, hw:hw + na], 1e-16)
                        rec = blkp.tile([P, na], f32, tag="rec")
                        nc.vector.reciprocal(rec[:], den[:])
                        xo = blkp.tile([P, hw], f32, tag="xo")
                        rec_b = bass.AP(rec.tensor, rec[:].offset,
                                        [rec[:].ap[0], [1, na], [0, chw]])
                        nc.vector.tensor_tensor(out=xo[:], in0=acc[:, 0:hw],
                                                in1=rec_b,
                                                op=mybir.AluOpType.mult)
                        if tau_t[layer] is not None:
                            nc.vector.tensor_tensor(
                                out=xo[:], in0=xo[:], in1=tau_t[layer][:],
                                op=mybir.AluOpType.add)
                        if layer < 2:
                            ng = blkp.tile([P, hw], f32, tag="ng")
                            nc.vector.tensor_scalar_min(ng[:], xo[:], 0.0)
                            en = blkp.tile([P, hw], f32, tag="en")
                            nc.scalar.activation(
                                en[:], ng[:],
                                mybir.ActivationFunctionType.Exp)
                            ps = blkp.tile([P, hw], f32, tag="ps")
                            nc.vector.tensor_scalar_max(ps[:], xo[:], 0.0)
                            xe = blkp.tile([P, hw], f16, tag="xe")
                            nc.vector.scalar_tensor_tensor(
                                out=xe[:], in0=en[:], scalar=-1.0,
                                in1=ps[:], op0=mybir.AluOpType.add,
                                op1=mybir.AluOpType.add)
                            nc.sync.dma_start(
                                xbuf[layer][blk * P:(blk + 1) * P, :], xe[:])
                        else:
                            # int8 quantize with per-row scale
                            rmax = blkp.tile([P, 1], f32, tag="rmax")
                            nc.vector.tensor_reduce(
                                rmax[:], xo[:], axis=mybir.AxisListType.X,
                                op=mybir.AluOpType.max,
                                apply_absolute_value=True)
                            rmx = blkp.tile([P, 1], f32, tag="rmx")
                            nc.vector.tensor_scalar_max(rmx[:], rmax[:],
                                                        1e-6)
                            rs = blkp.tile([P, 1], f32, tag="rs")
                            nc.vector.reciprocal(rs[:], rmx[:])
                            yqf = blkp.tile([P, OUT], f32, tag="yqf")
                            nc.vector.scalar_tensor_tensor(
                                out=yqf[:], in0=xo[:], scalar=127.0,
                                in1=bass.AP(rs.tensor, rs[:].offset,
                                            [rs[:].ap[0], [0, OUT]]),
                                op0=mybir.AluOpType.mult,
                                op1=mybir.AluOpType.mult)
                            yo = blkp.tile([P, OUT + 2], i8, tag="yo")
                            nc.vector.tensor_copy(yo[:, 0:OUT], yqf[:])
                            nc.vector.tensor_scalar_mul(
                                yo[:, OUT:OUT + 2].bitcast(f16), rmx[:],
                                1.0 / 127.0)
                            nc.sync.dma_start(
                                yq[blk * P:(blk + 1) * P, :], yo[:])
                    b0 += nb
                    offL += nL * 8
                    offT += nA * P

            nlayers = int(os.environ.get("K_LAYERS", "3"))
            do_edge = os.environ.get("K_EDGE", "1") == "1"
            nrep = int(os.environ.get("K_REPEAT", "1"))
            for _rep in range(nrep):
                for layer in range(nlayers):
                    node_phase(layer)
                    if do_edge:
                        edge_phase(layer)
            if nlayers < 3 or not do_edge:
                zt = blkp.tile([P, OUT + 2], i8, tag="ytouch")
                nc.gpsimd.memset(zt[:], 0.0)
                for blk in range(BPC):
                    nc.sync.dma_start(yq[blk * P:(blk + 1) * P, :], zt[:])
    nc.compile()
    return nc


# ---------------------------------------------------------------------------
# numpy mirror of the device pipeline (for plan/stream validation)
# ---------------------------------------------------------------------------

def numpy_pipeline(pl, x, Wcs, aWs, taus):
    cfg = pl["cfg"]
    perm = pl["perm"]
    xp = np.zeros((cfg.NPAD, cfg.IN_CH), np.float32)
    xp[perm] = x
    S = cfg.S
    nsc_blk = 4 * S
    for layer in range(3):
        hw = cfg.HID if layer < 2 else cfg.OUT_CH
        na = cfg.HEADS if layer < 2 else 1
        chw = cfg.CH if layer < 2 else cfg.OUT_CH
        h = (xp @ Wcs[layer]).astype(np.float32)   # [NPAD, hw+na]
        table = np.zeros((cfg.NPAD, EL), np.float16)
        table[:, 0:hw] = h[:, 0:hw].astype(np.float16)
        adst16 = h[:, hw:hw + na].astype(np.float16)
        aW = aWs[layer]
        xnew = np.zeros((cfg.NPAD, hw), np.float32)
        for c in range(cfg.NCORES):
            gidx, dstl, dtv = pl["gidx"][c], pl["dstl"][c], pl["dtv"][c]
            for blk in range(cfg.BPC):
                accw = np.zeros((P, hw + na), np.float32)
                adst_blk = adst16[c * cfg.NPC + blk * P:
                                  c * cfg.NPC + (blk + 1) * P]
                for s in range(nsc_blk):
                    sc = blk * nsc_blk + s
                    half = 0 if s < 2 * S else 1
                    w = (s % (2 * S)) // S
                    gi = gidx[sc] + (cfg.HALFN if half else 0)
                    rows = table[gi]                         # [P, EL] f16
                    # grouped reduce: heads of width hw//na (aW zero-padded)
                    asrc = (rows.astype(np.float32) * aW[None, :])[
                        :, 0:hw].reshape(P, na, hw // na).sum(-1)
                    dv = dtv[sc].astype(np.int64)
                    ad = np.where(dv[:, None] >= 0,
                                  adst_blk[np.maximum(dv, 0)].astype(
                                      np.float32), 0.0)
                    e = asrc + ad
                    e = np.maximum(e, NEG_SLOPE * e)
                    ee = np.exp(e).astype(np.float16)
                    msg = (rows[:, 0:hw] *
                           np.repeat(ee, chw, axis=1)).astype(np.float16)
                    sl = dstl[sc]
                    mrow = np.concatenate(
                        [msg.astype(np.float32), ee.astype(np.float32)], 1)
                    if s == 0:
                        indm = (np.arange(P)[None, :] == sl[:, None])
                        accw += indm.astype(np.float32).T @ mrow
                    else:
                        indm = (np.arange(64)[None, :] == sl[:, None])
                        accw[w * 64:(w + 1) * 64] += \
                            indm.astype(np.float32).T @ mrow
                rec = 1.0 / (accw[:, hw:hw + na] + 1e-16)
                xo = accw[:, 0:hw] * np.repeat(rec, chw, axis=1)
                if taus[layer] is not None:
                    xo = xo + taus[layer][None, :]
                if layer < 2:
                    xo = np.where(xo > 0, xo,
                                  np.exp(np.minimum(xo, 0)) - 1)
                xnew[c * cfg.NPC + blk * P:c * cfg.NPC + (blk + 1) * P] = \
                    xo.astype(np.float16).astype(np.float32)
        xp = xnew
    return xp[perm]


# ---------------------------------------------------------------------------
# public entry
# ---------------------------------------------------------------------------

_CACHE = {}


def _get_program(cfg_key, cfg, with_tau):
    key = (cfg_key, with_tau, cfg.S)
    if key not in _CACHE:
        _CACHE[key] = build_program(cfg, with_tau)
    return _CACHE[key]


def prep_inputs(cfg, inputs):
    """Host planning + per-core input maps. Returns (nc, in_maps, perm)."""
    x = np.asarray(inputs["x"], np.float32)
    edge_index = np.asarray(inputs["edge_index"])
    pl = plan(cfg, edge_index)
    streams = make_streams(pl)
    Wc0, aW0, tau0 = fold_weights(
        inputs["W0"], inputs["a_src0"], inputs["a_dst0"], inputs["bn_g0"],
        inputs["bn_b0"], inputs["bn_m0"], inputs["bn_v0"], inputs["b0"])
    Wc1, aW1, tau1 = fold_weights(
        inputs["W1"], inputs["a_src1"], inputs["a_dst1"], inputs["bn_g1"],
        inputs["bn_b1"], inputs["bn_m1"], inputs["bn_v1"], inputs["b1"])
    Wc2, aW2, tau2 = fold_weights(
        inputs["W2"], inputs["a_src2"], inputs["a_dst2"], bias=inputs["b2"])
    taus = [tau0, tau1, tau2]
    with_tau = tuple(bool(np.any(t != 0)) for t in taus)
    nc = _get_program("gat", cfg, with_tau)

    perm = pl["perm"]
    OFF = blob_layout(cfg)
    aW_row = np.concatenate([aW0, aW1, aW2]).astype(np.float16)
    xs = np.maximum(np.abs(x).max(axis=0), 1e-9) / 127.0  # per input channel
    wc16 = [(Wc0 * xs[:, None]).astype(np.float16).ravel(),
            Wc1.astype(np.float16).ravel(),
            Wc2.astype(np.float16).ravel()]
    in_maps = []
    for c in range(cfg.NCORES):
        xc = np.zeros((cfg.NPC, cfg.IN_CH), np.float32)
        mask = (perm >= c * cfg.NPC) & (perm < (c + 1) * cfg.NPC)
        origs = np.nonzero(mask)[0]
        xc[perm[origs] - c * cfg.NPC] = x[origs]
        xcT = np.ascontiguousarray(xc.T)
        s = streams[c]
        blob = np.concatenate([
            pack_x8(xcT, xs).ravel().view(np.float16),
            wc16[0], wc16[1], wc16[2], aW_row,
            s["idxL"].ravel().view(np.float16),
            s["idxH"].ravel().view(np.float16),
            s["dstl"].ravel().view(np.float16),
            s["dstlT"].ravel().view(np.float16),
        ])[None, :]
        assert blob.shape[1] == OFF["TOT"], (blob.shape, OFF["TOT"])
        m = dict(blob=blob)
        for li, t in enumerate(taus):
            if with_tau[li]:
                m[f"tau{li}"] = t[None, :].astype(np.float32)
        in_maps.append(m)
    return nc, in_maps, perm, (pl, [Wc0, Wc1, Wc2], [aW0, aW1, aW2], taus,
                               with_tau)


def unquant_y(results, ncores):
    """[core]{yq} (int8 rows [q(40)|f16 scale]) -> full [NPAD, OUT] f32."""
    outs = []
    for c in range(ncores):
        raw = np.ascontiguousarray(results[c]["yq"])
        nout = raw.shape[1] - 2
        q = raw[:, 0:nout].astype(np.float32)
        s = raw[:, nout:nout + 2].copy().view(np.float16).astype(np.float32)
        outs.append(q * s)
    return np.concatenate(outs, axis=0)


def run(cfg, inputs, trace=False):
    nc, in_maps, perm, _ = prep_inputs(cfg, inputs)
    res = run_bass_kernel_spmd(nc, in_maps, list(range(cfg.NCORES)),
                               trace=trace)
    out = unquant_y(res.results, cfg.NCORES)[perm]
    return out, res


def kernel(**inputs):
    cfg = Cfg(n=50000, e=800000)
    out, _ = run(cfg, inputs)
    return out.astype(np.float32)
